# revision 9
# baseline (speedup 1.0000x reference)
"""Trainium2 Bass kernel for causal multi-head attention with RoPE.

Full-input contract: kernel(**inputs) takes the unsharded tensors and
returns the full [B, S, D] output. Internally the work is sharded over
8 NeuronCores: cores 0-3 compute batch 0, cores 4-7 batch 1; within a
batch group each core owns 4 of the 16 heads (tensor-parallel over
heads). Each core computes its partial output-projection contribution
[S, D]; the host sums the 4 partials per batch and adds the biases
that commute with attention (wo_b, and wv_b which passes through the
softmax untouched because attention weights sum to 1).

v3: all operands fp16 (half the DMA/SBUF of fp32r at the same PE
rate), every DRAM tensor pre-packed on the host so each DMA moves
contiguous 4KB-per-partition pieces (big descriptors - the v2 lesson:
rearranging in the DMA shatters loads into 1KB descriptors and the
input stream takes 25us). x/Q/K/V all stay resident in SBUF. V is
projected first, then per-head QK-projection + RoPE is software-
pipelined against the previous head's attention so exp latency hides
under projection matmuls. Causal masking is done with a column
prefill of -30000 into PSUM for the fully-masked columns plus a tiny
128x128 triangle multiply on the diagonal tile, which keeps the big
DVE ops off the QK->exp->PV critical path. Scores exp in 2-bank waves
(one ACT instruction per 1024 columns). The softmax denominator is a
fp16 DVE accumulation plus one ones-matmul per query chunk.
"""

import os
import sys

sys.path.insert(0, "/opt/trn_rl_repo")

import numpy as np

B = 2
S = 2048
D = 2048
H = 16
DK = 128
N_CORES = 8
HPC = 4          # heads per core
E = HPC * DK     # 512: per-core slice of the model dim
KO = D // 128    # contraction chunks for the projections
NJ = S // 128    # key blocks
SC = 512         # attention query chunk
NI = S // SC     # query chunks
ISQRT_DK = 1.0 / np.sqrt(DK)

_CACHE = {}

last_exec_time_ns = None
last_results = None


def _build_program():
    import concourse.mybir as mybir
    import concourse.tile as tile
    from concourse import bacc

    dt = mybir.dt
    F32 = dt.float32
    F16 = dt.float16
    AF = mybir.ActivationFunctionType

    nc = bacc.Bacc(None, target_bir_lowering=False, debug=True)

    # all tensors host-packed: partition dim first, contiguous free dims
    xT = nc.dram_tensor("xT", [128, NI, KO, SC], F16, kind="ExternalInput")
    wqT = nc.dram_tensor("wqT", [128, KO, E], F16, kind="ExternalInput")
    wkT = nc.dram_tensor("wkT", [128, KO, E], F16, kind="ExternalInput")
    wvT = nc.dram_tensor("wvT", [128, KO, E], F16, kind="ExternalInput")
    woT = nc.dram_tensor("woT", [128, HPC, D], F16, kind="ExternalInput")
    bq = nc.dram_tensor("bq", [DK, HPC], F32, kind="ExternalInput")
    bk = nc.dram_tensor("bk", [DK, HPC], F32, kind="ExternalInput")
    cc2 = nc.dram_tensor("cc2", [DK, S], F16, kind="ExternalInput")
    sss = nc.dram_tensor("sss", [DK, S], F16, kind="ExternalInput")
    tri = nc.dram_tensor("tri", [128, 128], F16, kind="ExternalInput")
    negbig = nc.dram_tensor("negbig", [128, 384], F16, kind="ExternalInput")
    ones = nc.dram_tensor("ones", [128, 128], F16, kind="ExternalInput")
    out = nc.dram_tensor("out", [S, D], F16, kind="ExternalOutput")

    with tile.TileContext(nc) as tc:
        with (
            tc.tile_pool(name="const", bufs=1) as cpool,
            tc.tile_pool(name="res", bufs=1) as respool,
            tc.tile_pool(name="wqk", bufs=1) as wqkpool,
            tc.tile_pool(name="bwork", bufs=3) as p2pool,
            tc.tile_pool(name="bacc", bufs=2) as accpool,
            tc.tile_pool(name="bli", bufs=2) as lipool,
            tc.tile_pool(name="xres", bufs=1) as xpool,
        ):
            # ---- resident tiles ----
            x_sb = xpool.tile([128, NI, KO, SC], F16, name="x_sb")
            vt = respool.tile([128, NJ, E], F16, name="vt")
            qres = respool.tile([DK, HPC, S], F16, name="qres")
            kres = respool.tile([DK, HPC, S], F16, name="kres")
            aores = respool.tile([DK, HPC, S], F16, name="aores")
            wq_sb = wqkpool.tile([128, KO, E], F16, name="wq_sb")
            wk_sb = wqkpool.tile([128, KO, E], F16, name="wk_sb")

            # ---- input streams, need-ordered ----
            # wv split across the scalar/gpsimd queues, x chunks on sync;
            # everything is DRAM-contiguous per partition (128 descriptors)
            wv_ctx = tc.tile_pool(name="wv", bufs=1)
            wvpool = wv_ctx.__enter__()
            wv_sb = wvpool.tile([128, KO, E], F16, name="wv_sb")
            for g in range(4):
                q = nc.scalar if g < 2 else nc.gpsimd
                q.dma_start(
                    wv_sb[:, g * 4 : (g + 1) * 4, :],
                    wvT[:, g * 4 : (g + 1) * 4, :],
                )
            for si in range(NI):
                nc.sync.dma_start(x_sb[:, si, :, :], xT[:, si, :, :])
            # constants
            bq_sb = cpool.tile([DK, HPC], F32, name="bq_sb")
            nc.gpsimd.dma_start(bq_sb[:], bq[:])
            bk_sb = cpool.tile([DK, HPC], F32, name="bk_sb")
            nc.gpsimd.dma_start(bk_sb[:], bk[:])
            cc2_sb = cpool.tile([DK, S], F16, name="cc2_sb")
            nc.gpsimd.dma_start(cc2_sb[:], cc2[:])
            sss_sb = cpool.tile([DK, S], F16, name="sss_sb")
            nc.gpsimd.dma_start(sss_sb[:], sss[:])
            tri_sb = cpool.tile([128, 128], F16, name="tri_sb")
            nc.gpsimd.dma_start(tri_sb[:], tri[:])
            neg_sb = cpool.tile([128, 384], F16, name="neg_sb")
            nc.gpsimd.dma_start(neg_sb[:], negbig[:])
            ones_sb = cpool.tile([128, 128], F16, name="ones_sb")
            nc.gpsimd.dma_start(ones_sb[:], ones[:])
            # weights for Q/K projections (needed after V completes)
            for wsb, wdram in ((wk_sb, wkT), (wq_sb, wqT)):
                for g in range(4):
                    q = nc.scalar if g % 2 else nc.gpsimd
                    q.dma_start(
                        wsb[:, g * 4 : (g + 1) * 4, :],
                        wdram[:, g * 4 : (g + 1) * 4, :],
                    )

            # ---------- Phase V: value projection, vt resident ----------
            vps_ctx = tc.tile_pool(name="vps", bufs=2, space="PSUM")
            vpspool = vps_ctx.__enter__()
            for si in range(NI):
                for jj in range(4):
                    pv = vpspool.tile([128, E], F32, tag="pv")
                    jsl = slice(jj * 128, (jj + 1) * 128)
                    for k in range(KO):
                        nc.tensor.matmul(
                            pv[:],
                            x_sb[:, si, k, jsl],
                            wv_sb[:, k, :],
                            start=(k == 0),
                            stop=(k == KO - 1),
                        )
                    nc.vector.tensor_copy(vt[:, si * 4 + jj, :], pv[:])
            vps_ctx.__exit__(None, None, None)
            wv_ctx.__exit__(None, None, None)

            # ---------- interleaved QK projection + attention ----------
            qkps_ctx = tc.tile_pool(name="qkps", bufs=2, space="PSUM")
            pqpool = qkps_ctx.__enter__()
            bps_ctx = tc.tile_pool(name="bps", bufs=2, space="PSUM")
            ps2pool = bps_ctx.__enter__()
            bpo_ctx = tc.tile_pool(name="bpo", bufs=1, space="PSUM")
            popool = bpo_ctx.__enter__()
            bpl_ctx = tc.tile_pool(name="bpl", bufs=1, space="PSUM")
            plpool = bpl_ctx.__enter__()
            st_ctx = tc.tile_pool(name="stw", bufs=2)
            stpool = st_ctx.__enter__()
            sw_ctx = tc.tile_pool(name="sww", bufs=2)
            swpool = sw_ctx.__enter__()

            def qkproj_chunks(h):
                """8 emission chunks: (k, nsl0), (q, nsl0), (k, nsl1), ...
                RoPE runs on half-strips [128, 1024] after odd nsl chunks:
                cross-partition swaps + cos-mul on DVE, sin-mul + add on
                gpsimd."""
                chunks = []
                state = {}
                for nsl in range(4):
                    for wsb, bsb, dst in (
                        (wk_sb, bk_sb, kres),
                        (wq_sb, bq_sb, qres),
                    ):
                        def emit(nsl=nsl, wsb=wsb, bsb=bsb, dst=dst):
                            sl = slice(nsl * SC, (nsl + 1) * SC)
                            pq = pqpool.tile([128, SC], F32, tag="pq")
                            for k in range(KO):
                                nc.tensor.matmul(
                                    pq[:],
                                    wsb[:, k, h * DK : (h + 1) * DK],
                                    x_sb[:, nsl, k, :],
                                    start=(k == 0),
                                    stop=(k == KO - 1),
                                )
                            half = nsl // 2
                            part = nsl % 2
                            if part == 0:
                                st = stpool.tile(
                                    [128, 2 * SC], F16, tag="st", name="st"
                                )
                                state[(dst is qres, half)] = st
                            else:
                                st = state[(dst is qres, half)]
                            nc.scalar.activation(
                                st[:, part * SC : (part + 1) * SC],
                                pq[:], AF.Identity,
                                bias=bsb[:, h : h + 1],
                            )
                            if part == 1:
                                hsl = slice(half * 2 * SC, (half + 1) * 2 * SC)
                                sw = swpool.tile([128, 2 * SC], F16, tag="sw")
                                nc.vector.tensor_copy(
                                    sw[0:64, :], st[64:128, :]
                                )
                                nc.vector.tensor_copy(
                                    sw[64:128, :], st[0:64, :]
                                )
                                nc.gpsimd.tensor_mul(
                                    sw[:], sw[:], sss_sb[:, hsl]
                                )
                                nc.vector.tensor_mul(
                                    dst[:, h, hsl], st[:], cc2_sb[:, hsl]
                                )
                                nc.gpsimd.tensor_add(
                                    dst[:, h, hsl], dst[:, h, hsl], sw[:]
                                )
                        chunks.append(emit)
                return chunks

            def attn_chunks(h):
                """4 emission chunks, one per query chunk ic."""
                chunks = []
                for ic in range(NI):
                    def emit(ic=ic):
                        njc = 4 * ic + 4
                        i0 = ic * SC
                        po = popool.tile([128, SC], F32, tag="po")
                        acc = accpool.tile([128, SC], F16, tag="acc")
                        pend = []

                        def flush(wave, ws2):
                            nw = len(wave)
                            p2 = p2pool.tile([128, 2, SC], F16, tag="p2")
                            nc.scalar.activation(
                                p2[:, 0:nw, :], ws2[:, 0:nw, :], AF.Exp,
                                scale=float(ISQRT_DK),
                            )
                            for j, (jc, t, cs) in enumerate(wave):
                                if t >= 0:
                                    # in-tile causal triangle
                                    nc.vector.tensor_mul(
                                        p2[:, j, cs : cs + 128],
                                        p2[:, j, cs : cs + 128],
                                        tri_sb[:],
                                    )
                                if jc == 0:
                                    nc.vector.tensor_copy(acc[:], p2[:, j, :])
                                else:
                                    nc.vector.tensor_add(
                                        acc[:], acc[:], p2[:, j, :]
                                    )
                            pend.append((p2, wave))

                        def drain_pv():
                            p2, wave = pend.pop(0)
                            for j, (jc, t, cs) in enumerate(wave):
                                nc.tensor.matmul(
                                    po[:, cs:],
                                    vt[:, jc, h * DK : (h + 1) * DK],
                                    p2[:, j, cs:],
                                    start=(jc == 0),
                                    stop=(jc == njc - 1),
                                )

                        wave, ws2 = [], None
                        for jc in range(njc):
                            t = jc - 4 * ic
                            cs = 128 * t if t >= 0 else 0
                            if not wave:
                                ws2 = ps2pool.tile([128, 2, SC], F32, tag="ps2")
                            if cs > 0:
                                # fully-masked leading columns: prefill with
                                # -30000 so the full-strip exp sees zeros
                                nc.vector.tensor_copy(
                                    ws2[:, len(wave), 0:cs], neg_sb[:, 0:cs]
                                )
                            nc.tensor.matmul(
                                ws2[:, len(wave), cs:],
                                kres[:, h, jc * 128 : (jc + 1) * 128],
                                qres[:, h, i0 + cs : i0 + SC],
                                start=True,
                                stop=True,
                            )
                            wave.append((jc, t, cs))
                            if len(wave) == 2:
                                flush(wave, ws2)
                                wave, ws2 = [], None
                                if len(pend) > 1:
                                    drain_pv()
                        while pend:
                            drain_pv()

                        pl = plpool.tile([128, SC], F32, tag="pl")
                        nc.tensor.matmul(
                            pl[:], ones_sb[:], acc[:], start=True, stop=True
                        )
                        li = lipool.tile([128, SC], F32, tag="li")
                        nc.vector.reciprocal_approx_fast(li[:], pl[:])
                        nc.vector.tensor_mul(
                            aores[:, h, i0 : i0 + SC], po[:], li[:]
                        )
                    chunks.append(emit)
                return chunks

            # schedule: qkproj(0), then per head: attention(h) with
            # qkproj(h+1) chunks slotted after each query chunk
            for ch in qkproj_chunks(0):
                ch()
            for h in range(HPC - 1):
                nxt = qkproj_chunks(h + 1)
                at = attn_chunks(h)
                for ic in range(NI):
                    at[ic]()
                    for ch in nxt[2 * ic : 2 * ic + 2]:
                        ch()

            # projection scratch done; swap for output weights, prefetched
            # during the last head's attention
            sw_ctx.__exit__(None, None, None)
            st_ctx.__exit__(None, None, None)
            wo_ctx = tc.tile_pool(name="wo", bufs=1)
            wopool = wo_ctx.__enter__()
            wo_sb = wopool.tile([128, HPC, D], F16, name="wo_sb")
            for g in range(4):
                (nc.sync if g % 2 == 0 else nc.gpsimd).dma_start(
                    wo_sb[:, g, :], woT[:, g, :]
                )
            for ch in attn_chunks(HPC - 1):
                ch()

            bpl_ctx.__exit__(None, None, None)
            bpo_ctx.__exit__(None, None, None)
            bps_ctx.__exit__(None, None, None)
            qkps_ctx.__exit__(None, None, None)

            # ---------- Phase C: output projection (partial sums) ----------
            with (
                tc.tile_pool(name="cob", bufs=2) as obpool,
                tc.tile_pool(name="cps", bufs=2, space="PSUM") as cpspool,
            ):
                for ii in range(S // 128):
                    isl = slice(ii * 128, (ii + 1) * 128)
                    ob = obpool.tile([128, D], F16, tag="ob")
                    for half in range(2):
                        pc = cpspool.tile([128, 2, 512], F32, tag="pc")
                        for ec in range(HPC):
                            for f2 in range(2):
                                fc = half * 2 + f2
                                nc.tensor.matmul(
                                    pc[:, f2, :],
                                    aores[:, ec, isl],
                                    wo_sb[:, ec, fc * 512 : (fc + 1) * 512],
                                    start=(ec == 0),
                                    stop=(ec == HPC - 1),
                                )
                        for f2 in range(2):
                            fc = half * 2 + f2
                            osl = slice(fc * 512, (fc + 1) * 512)
                            if f2 == 0:
                                nc.vector.tensor_copy(ob[:, osl], pc[:, f2, :])
                            else:
                                nc.scalar.activation(
                                    ob[:, osl], pc[:, f2, :], AF.Copy
                                )
                    (nc.sync if ii % 2 == 0 else nc.scalar).dma_start(
                        out[isl, :], ob[:]
                    )
            wo_ctx.__exit__(None, None, None)

    nc.compile()
    return nc


def _rope_tables():
    inv_freq = 1.0 / (10000.0 ** (np.arange(0, DK, 2, dtype=np.float64) / DK))
    pos = np.arange(S, dtype=np.float64)
    freqs = pos[:, None] * inv_freq[None, :]  # [S, DK/2]
    cos_t = np.cos(freqs).T.astype(np.float16)  # [64, S]
    sin_t = np.sin(freqs).T.astype(np.float16)
    cc2 = np.ascontiguousarray(np.concatenate([cos_t, cos_t], axis=0))
    sss = np.ascontiguousarray(np.concatenate([-sin_t, sin_t], axis=0))
    return cc2, sss


def _pack_pke(w16):
    """[D, E] -> [128, KO, E] with partition dim first, contiguous."""
    return np.ascontiguousarray(
        w16.reshape(KO, 128, E).transpose(1, 0, 2)
    )


def kernel(
    x, wq_w, wq_b, wk_w, wk_b, wv_w, wv_b, wo_w, wo_b
) -> np.ndarray:
    global last_exec_time_ns, last_results
    from concourse.bass_utils import run_bass_kernel_spmd

    if "nc" not in _CACHE:
        _CACHE["nc"] = _build_program()
    nc = _CACHE["nc"]

    x = np.asarray(x, dtype=np.float32)
    wq_w = np.asarray(wq_w, dtype=np.float32)
    wk_w = np.asarray(wk_w, dtype=np.float32)
    wv_w = np.asarray(wv_w, dtype=np.float32)
    wo_w = np.asarray(wo_w, dtype=np.float32)
    wq_b = np.asarray(wq_b, dtype=np.float32)
    wk_b = np.asarray(wk_b, dtype=np.float32)
    wv_b = np.asarray(wv_b, dtype=np.float32)
    wo_b = np.asarray(wo_b, dtype=np.float32)

    cc2, sss = _rope_tables()
    r_idx = np.arange(128)[:, None]
    c_idx = np.arange(128)[None, :]
    tri = np.ascontiguousarray((r_idx <= c_idx).astype(np.float16))
    negbig = np.full((128, 384), -30000.0, dtype=np.float16)
    ones = np.ones((128, 128), dtype=np.float16)
    # within each head, pack d-rows as [even dims; odd dims]
    perm = np.concatenate([np.arange(0, DK, 2), np.arange(1, DK, 2)])

    # x: [S, D] -> xT [D, S] -> [128, NI(si), KO(k), SC] contiguous
    xT_b = [
        np.ascontiguousarray(
            x[b].T.astype(np.float16)
            .reshape(KO, 128, NI, SC)
            .transpose(1, 2, 0, 3)
        )
        for b in range(B)
    ]

    in_maps = []
    for c in range(N_CORES):
        b = c // (N_CORES // B)
        g = c % (N_CORES // B)
        es = g * E

        def pack_qk(w):
            rows = w[es : es + E]  # [E, D]
            blocks = [
                rows[h0 * DK : (h0 + 1) * DK][perm] for h0 in range(HPC)
            ]
            return _pack_pke(
                np.concatenate(blocks, axis=0).T.astype(np.float16)
            )

        def pack_bias(bvec):
            sl = bvec[es : es + E].reshape(HPC, DK)
            return np.ascontiguousarray(sl[:, perm].T)  # [DK, HPC]

        in_maps.append(
            {
                "xT": xT_b[b],
                "wqT": pack_qk(wq_w),
                "wkT": pack_qk(wk_w),
                "wvT": _pack_pke(wv_w[es : es + E].T.astype(np.float16)),
                "woT": np.ascontiguousarray(
                    wo_w[:, es : es + E].T.astype(np.float16)
                    .reshape(HPC, 128, D)
                    .transpose(1, 0, 2)
                ),
                "bq": pack_bias(wq_b),
                "bk": pack_bias(wk_b),
                "cc2": cc2,
                "sss": sss,
                "tri": tri,
                "negbig": negbig,
                "ones": ones,
            }
        )

    trace = bool(os.environ.get("MHA_TRACE"))
    res = run_bass_kernel_spmd(
        nc, in_maps, list(range(N_CORES)), trace=trace
    )
    last_exec_time_ns = res.exec_time_ns
    last_results = res

    # host-side gather: sum partials per batch, add biases that commute
    # with attention (softmax rows sum to 1, so wv_b passes straight
    # through to the output projection)
    const_bias = wo_b + wo_w @ wv_b  # [D]
    out = np.empty((B, S, D), dtype=np.float32)
    gpb = N_CORES // B
    for b in range(B):
        acc = res.results[b * gpb]["out"].astype(np.float32)
        for c in range(b * gpb + 1, (b + 1) * gpb):
            acc += res.results[c]["out"].astype(np.float32)
        out[b] = acc + const_bias[None, :]
    return out


# revision 14
# speedup vs baseline: 1.2969x; 1.2969x over previous
"""Trainium2 Bass kernel for causal multi-head attention with RoPE.

Full-input contract: kernel(**inputs) takes the unsharded tensors and
returns the full [B, S, D] output. Internally the work is sharded over
8 NeuronCores: cores 0-3 compute batch 0, cores 4-7 batch 1; within a
batch group each core owns 4 of the 16 heads (tensor-parallel over
heads). Each core computes its partial output-projection contribution
[S, D]; the host sums the 4 partials per batch and adds the biases
that commute with attention (wo_b, and wv_b which passes through the
softmax untouched because attention weights sum to 1).

v3: all operands fp16 (half the DMA/SBUF of fp32r at the same PE
rate), every DRAM tensor pre-packed on the host so each DMA moves
contiguous 4KB-per-partition pieces (big descriptors - the v2 lesson:
rearranging in the DMA shatters loads into 1KB descriptors and the
input stream takes 25us). x/Q/K/V all stay resident in SBUF. V is
projected first, then per-head QK-projection + RoPE is software-
pipelined against the previous head's attention so exp latency hides
under projection matmuls. Causal masking is done with a column
prefill of -30000 into PSUM for the fully-masked columns plus a tiny
128x128 triangle multiply on the diagonal tile, which keeps the big
DVE ops off the QK->exp->PV critical path. Scores exp in 2-bank waves
(one ACT instruction per 1024 columns). The softmax denominator is a
fp16 DVE accumulation plus one ones-matmul per query chunk.
"""

import os
import sys

sys.path.insert(0, "/opt/trn_rl_repo")

import numpy as np

B = 2
S = 2048
D = 2048
H = 16
DK = 128
N_CORES = 8
HPC = 4          # heads per core
E = HPC * DK     # 512: per-core slice of the model dim
KO = D // 128    # contraction chunks for the projections
NJ = S // 128    # key blocks
SC = 512         # attention query chunk
NI = S // SC     # query chunks
ISQRT_DK = 1.0 / np.sqrt(DK)

_CACHE = {}

last_exec_time_ns = None
last_results = None


def _build_program():
    import concourse.mybir as mybir
    import concourse.tile as tile
    from concourse import bacc

    dt = mybir.dt
    F32 = dt.float32
    F16 = dt.float16
    AF = mybir.ActivationFunctionType

    nc = bacc.Bacc(None, target_bir_lowering=False, debug=True)

    # all tensors host-packed: partition dim first, contiguous free dims
    xT = nc.dram_tensor("xT", [128, NI, KO, SC], F16, kind="ExternalInput")
    wqT = nc.dram_tensor("wqT", [128, KO, E], F16, kind="ExternalInput")
    wkT = nc.dram_tensor("wkT", [128, KO, E], F16, kind="ExternalInput")
    wvT = nc.dram_tensor("wvT", [128, KO, E], F16, kind="ExternalInput")
    woT = nc.dram_tensor("woT", [128, HPC, D], F16, kind="ExternalInput")
    bq = nc.dram_tensor("bq", [DK, HPC], F32, kind="ExternalInput")
    bk = nc.dram_tensor("bk", [DK, HPC], F32, kind="ExternalInput")
    cc2 = nc.dram_tensor("cc2", [DK, S], F16, kind="ExternalInput")
    sss = nc.dram_tensor("sss", [DK, S], F16, kind="ExternalInput")
    tri = nc.dram_tensor("tri", [128, 128], F16, kind="ExternalInput")
    ones = nc.dram_tensor("ones", [128, 128], F16, kind="ExternalInput")
    out = nc.dram_tensor("out", [S, D], F16, kind="ExternalOutput")

    with tile.TileContext(nc) as tc:
        with (
            tc.tile_pool(name="const", bufs=1) as cpool,
            tc.tile_pool(name="res", bufs=1) as respool,
            tc.tile_pool(name="wqk", bufs=1) as wqkpool,
            tc.tile_pool(name="bwork", bufs=3) as p2pool,
            tc.tile_pool(name="bacc", bufs=2) as accpool,
            tc.tile_pool(name="bli", bufs=2) as lipool,
            tc.tile_pool(name="xres", bufs=1) as xpool,
        ):
            # ---- resident tiles ----
            x_sb = xpool.tile([128, NI, KO, SC], F16, name="x_sb")
            vt = respool.tile([128, NJ, E], F16, name="vt")
            qres = respool.tile([DK, HPC, S], F16, name="qres")
            kres = respool.tile([DK, HPC, S], F16, name="kres")
            aores = respool.tile([DK, HPC, S], F16, name="aores")
            wq_sb = wqkpool.tile([128, KO, E], F16, name="wq_sb")
            wk_sb = wqkpool.tile([128, KO, E], F16, name="wk_sb")

            # ---- input streams, need-ordered ----
            # wv split across the scalar/gpsimd queues, x chunks on sync;
            # everything is DRAM-contiguous per partition (128 descriptors)
            wv_ctx = tc.tile_pool(name="wv", bufs=1)
            wvpool = wv_ctx.__enter__()
            wv_sb = wvpool.tile([128, KO, E], F16, name="wv_sb")
            for g in range(4):
                q = nc.scalar if g % 2 == 0 else nc.gpsimd
                q.dma_start(
                    wv_sb[:, g * 4 : (g + 1) * 4, :],
                    wvT[:, g * 4 : (g + 1) * 4, :],
                )
            xq = [nc.sync, nc.scalar, nc.gpsimd]
            for i in range(8):
                si, kh = i // 2, i % 2
                xq[i % 3].dma_start(
                    x_sb[:, si, kh * 8 : (kh + 1) * 8, :],
                    xT[:, si, kh * 8 : (kh + 1) * 8, :],
                )
            # constants
            bq_sb = cpool.tile([DK, HPC], F32, name="bq_sb")
            nc.gpsimd.dma_start(bq_sb[:], bq[:])
            bk_sb = cpool.tile([DK, HPC], F32, name="bk_sb")
            nc.gpsimd.dma_start(bk_sb[:], bk[:])
            cc2_sb = cpool.tile([DK, S], F16, name="cc2_sb")
            nc.gpsimd.dma_start(cc2_sb[:], cc2[:])
            sss_sb = cpool.tile([DK, S], F16, name="sss_sb")
            nc.gpsimd.dma_start(sss_sb[:], sss[:])
            tri_sb = cpool.tile([128, 128], F16, name="tri_sb")
            nc.gpsimd.dma_start(tri_sb[:], tri[:])
            ones_sb = cpool.tile([128, 128], F16, name="ones_sb")
            nc.gpsimd.dma_start(ones_sb[:], ones[:])
            # weights for Q/K projections (needed after V completes)
            for wsb, wdram in ((wk_sb, wkT), (wq_sb, wqT)):
                for g in range(4):
                    q = nc.scalar if g % 2 else nc.gpsimd
                    q.dma_start(
                        wsb[:, g * 4 : (g + 1) * 4, :],
                        wdram[:, g * 4 : (g + 1) * 4, :],
                    )

            # ---------- Phase V: value projection, vt resident ----------
            vps_ctx = tc.tile_pool(name="vps", bufs=2, space="PSUM")
            vpspool = vps_ctx.__enter__()
            for si in range(NI):
                for jj in range(4):
                    pv = vpspool.tile([128, E], F32, tag="pv")
                    jsl = slice(jj * 128, (jj + 1) * 128)
                    for k in range(KO):
                        nc.tensor.matmul(
                            pv[:],
                            x_sb[:, si, k, jsl],
                            wv_sb[:, k, :],
                            start=(k == 0),
                            stop=(k == KO - 1),
                        )
                    nc.vector.tensor_copy(vt[:, si * 4 + jj, :], pv[:])
            vps_ctx.__exit__(None, None, None)
            wv_ctx.__exit__(None, None, None)

            # ---------- interleaved QK projection + attention ----------
            qkps_ctx = tc.tile_pool(name="qkps", bufs=2, space="PSUM")
            pqpool = qkps_ctx.__enter__()
            bps_ctx = tc.tile_pool(name="bps", bufs=2, space="PSUM")
            ps2pool = bps_ctx.__enter__()
            bpo_ctx = tc.tile_pool(name="bpo", bufs=1, space="PSUM")
            popool = bpo_ctx.__enter__()
            bpl_ctx = tc.tile_pool(name="bpl", bufs=1, space="PSUM")
            plpool = bpl_ctx.__enter__()
            st_ctx = tc.tile_pool(name="stw", bufs=2)
            stpool = st_ctx.__enter__()
            sw_ctx = tc.tile_pool(name="sww", bufs=2)
            swpool = sw_ctx.__enter__()

            def qkproj_chunks(h):
                """8 emission chunks: (k, nsl0), (q, nsl0), (k, nsl1), ...
                RoPE per [128, 512] chunk: cross-partition swaps + cos-mul
                on DVE, sin-mul + add on gpsimd (small ops so neither queue
                blocks the attention pipeline for long)."""
                chunks = []
                for nsl in range(4):
                    for wsb, bsb, dst in (
                        (wk_sb, bk_sb, kres),
                        (wq_sb, bq_sb, qres),
                    ):
                        def emit(nsl=nsl, wsb=wsb, bsb=bsb, dst=dst):
                            sl = slice(nsl * SC, (nsl + 1) * SC)
                            pq = pqpool.tile([128, SC], F32, tag="pq")
                            for k in range(KO):
                                nc.tensor.matmul(
                                    pq[:],
                                    wsb[:, k, h * DK : (h + 1) * DK],
                                    x_sb[:, nsl, k, :],
                                    start=(k == 0),
                                    stop=(k == KO - 1),
                                )
                            st = stpool.tile([128, SC], F16, tag="st")
                            nc.scalar.activation(
                                st[:], pq[:], AF.Identity,
                                bias=bsb[:, h : h + 1],
                            )
                            sw = swpool.tile([128, SC], F16, tag="sw")
                            nc.vector.tensor_copy(sw[0:64, :], st[64:128, :])
                            nc.vector.tensor_copy(sw[64:128, :], st[0:64, :])
                            nc.gpsimd.tensor_mul(sw[:], sw[:], sss_sb[:, sl])
                            nc.vector.tensor_mul(
                                dst[:, h, sl], st[:], cc2_sb[:, sl]
                            )
                            nc.gpsimd.tensor_add(
                                dst[:, h, sl], dst[:, h, sl], sw[:]
                            )
                        chunks.append(emit)
                return chunks

            def attn_chunks(h):
                """4 emission chunks, one per query chunk ic."""
                chunks = []
                for ic in range(NI):
                    def emit(ic=ic):
                        njc = 4 * ic + 4
                        i0 = ic * SC
                        po = popool.tile([128, SC], F32, tag="po")
                        acc = accpool.tile([128, SC], F16, tag="acc")
                        pend = []

                        def flush(wave, ws2):
                            p2 = p2pool.tile([128, 2, SC], F16, tag="p2")
                            if wave[-1][1] >= 0:
                                # diagonal band: exp only the valid columns
                                for j, (jc, t, cs) in enumerate(wave):
                                    nc.scalar.activation(
                                        p2[:, j, cs:], ws2[:, j, cs:],
                                        AF.Exp, scale=float(ISQRT_DK),
                                    )
                            else:
                                nc.scalar.activation(
                                    p2[:, 0:2, :], ws2[:, 0:2, :], AF.Exp,
                                    scale=float(ISQRT_DK),
                                )
                            for j, (jc, t, cs) in enumerate(wave):
                                if t >= 0:
                                    # in-tile causal triangle
                                    nc.vector.tensor_mul(
                                        p2[:, j, cs : cs + 128],
                                        p2[:, j, cs : cs + 128],
                                        tri_sb[:],
                                    )
                                if jc == 0:
                                    nc.vector.tensor_copy(acc[:], p2[:, j, :])
                                else:
                                    nc.vector.tensor_add(
                                        acc[:, cs:], acc[:, cs:],
                                        p2[:, j, cs:],
                                    )
                            pend.append((p2, wave))

                        def drain_pv():
                            p2, wave = pend.pop(0)
                            for j, (jc, t, cs) in enumerate(wave):
                                nc.tensor.matmul(
                                    po[:, cs:],
                                    vt[:, jc, h * DK : (h + 1) * DK],
                                    p2[:, j, cs:],
                                    start=(jc == 0),
                                    stop=(jc == njc - 1),
                                )

                        wave, ws2 = [], None
                        for jc in range(njc):
                            t = jc - 4 * ic
                            cs = 128 * t if t >= 0 else 0
                            if not wave:
                                ws2 = ps2pool.tile([128, 2, SC], F32, tag="ps2")
                            nc.tensor.matmul(
                                ws2[:, len(wave), cs:],
                                kres[:, h, jc * 128 : (jc + 1) * 128],
                                qres[:, h, i0 + cs : i0 + SC],
                                start=True,
                                stop=True,
                            )
                            wave.append((jc, t, cs))
                            if len(wave) == 2:
                                flush(wave, ws2)
                                wave, ws2 = [], None
                                if len(pend) > 1:
                                    drain_pv()
                        while pend:
                            drain_pv()

                        pl = plpool.tile([128, SC], F32, tag="pl")
                        nc.tensor.matmul(
                            pl[:], ones_sb[:], acc[:], start=True, stop=True
                        )
                        li = lipool.tile([128, SC], F32, tag="li")
                        nc.vector.reciprocal_approx_fast(li[:], pl[:])
                        nc.vector.tensor_mul(
                            aores[:, h, i0 : i0 + SC], po[:], li[:]
                        )
                    chunks.append(emit)
                return chunks

            # schedule: qkproj(0), then per head: attention(h) with
            # qkproj(h+1) chunks slotted after each query chunk
            for ch in qkproj_chunks(0):
                ch()
            for h in range(HPC - 1):
                nxt = qkproj_chunks(h + 1)
                at = attn_chunks(h)
                for ic in range(NI):
                    at[ic]()
                    for ch in nxt[2 * ic : 2 * ic + 2]:
                        ch()

            # projection scratch done; swap for output weights, prefetched
            # during the last head's attention
            sw_ctx.__exit__(None, None, None)
            st_ctx.__exit__(None, None, None)
            wo_ctx = tc.tile_pool(name="wo", bufs=1)
            wopool = wo_ctx.__enter__()
            wo_sb = wopool.tile([128, HPC, D], F16, name="wo_sb")
            for g in range(4):
                (nc.sync if g % 2 == 0 else nc.gpsimd).dma_start(
                    wo_sb[:, g, :], woT[:, g, :]
                )
            for ch in attn_chunks(HPC - 1):
                ch()

            bpl_ctx.__exit__(None, None, None)
            bpo_ctx.__exit__(None, None, None)
            bps_ctx.__exit__(None, None, None)
            qkps_ctx.__exit__(None, None, None)

            # ---------- Phase C: output projection (partial sums) ----------
            with (
                tc.tile_pool(name="cob", bufs=2) as obpool,
                tc.tile_pool(name="cps", bufs=2, space="PSUM") as cpspool,
            ):
                for ii in range(S // 128):
                    isl = slice(ii * 128, (ii + 1) * 128)
                    ob = obpool.tile([128, D], F16, tag="ob")
                    for half in range(2):
                        pc = cpspool.tile([128, 2, 512], F32, tag="pc")
                        for ec in range(HPC):
                            for f2 in range(2):
                                fc = half * 2 + f2
                                nc.tensor.matmul(
                                    pc[:, f2, :],
                                    aores[:, ec, isl],
                                    wo_sb[:, ec, fc * 512 : (fc + 1) * 512],
                                    start=(ec == 0),
                                    stop=(ec == HPC - 1),
                                )
                        for f2 in range(2):
                            fc = half * 2 + f2
                            osl = slice(fc * 512, (fc + 1) * 512)
                            if f2 == 0:
                                nc.vector.tensor_copy(ob[:, osl], pc[:, f2, :])
                            else:
                                nc.scalar.activation(
                                    ob[:, osl], pc[:, f2, :], AF.Copy
                                )
                    (nc.sync if ii % 2 == 0 else nc.scalar).dma_start(
                        out[isl, :], ob[:]
                    )
            wo_ctx.__exit__(None, None, None)

    nc.compile()
    return nc


def _rope_tables():
    inv_freq = 1.0 / (10000.0 ** (np.arange(0, DK, 2, dtype=np.float64) / DK))
    pos = np.arange(S, dtype=np.float64)
    freqs = pos[:, None] * inv_freq[None, :]  # [S, DK/2]
    cos_t = np.cos(freqs).T.astype(np.float16)  # [64, S]
    sin_t = np.sin(freqs).T.astype(np.float16)
    cc2 = np.ascontiguousarray(np.concatenate([cos_t, cos_t], axis=0))
    sss = np.ascontiguousarray(np.concatenate([-sin_t, sin_t], axis=0))
    return cc2, sss


def _pack_pke(w16):
    """[D, E] -> [128, KO, E] with partition dim first, contiguous."""
    return np.ascontiguousarray(
        w16.reshape(KO, 128, E).transpose(1, 0, 2)
    )


def kernel(
    x, wq_w, wq_b, wk_w, wk_b, wv_w, wv_b, wo_w, wo_b
) -> np.ndarray:
    global last_exec_time_ns, last_results
    from concourse.bass_utils import run_bass_kernel_spmd

    if "nc" not in _CACHE:
        _CACHE["nc"] = _build_program()
    nc = _CACHE["nc"]

    x = np.asarray(x, dtype=np.float32)
    wq_w = np.asarray(wq_w, dtype=np.float32)
    wk_w = np.asarray(wk_w, dtype=np.float32)
    wv_w = np.asarray(wv_w, dtype=np.float32)
    wo_w = np.asarray(wo_w, dtype=np.float32)
    wq_b = np.asarray(wq_b, dtype=np.float32)
    wk_b = np.asarray(wk_b, dtype=np.float32)
    wv_b = np.asarray(wv_b, dtype=np.float32)
    wo_b = np.asarray(wo_b, dtype=np.float32)

    cc2, sss = _rope_tables()
    r_idx = np.arange(128)[:, None]
    c_idx = np.arange(128)[None, :]
    tri = np.ascontiguousarray((r_idx <= c_idx).astype(np.float16))
    ones = np.ones((128, 128), dtype=np.float16)
    # within each head, pack d-rows as [even dims; odd dims]
    perm = np.concatenate([np.arange(0, DK, 2), np.arange(1, DK, 2)])

    # x: [S, D] -> xT [D, S] -> [128, NI(si), KO(k), SC] contiguous
    xT_b = [
        np.ascontiguousarray(
            x[b].T.astype(np.float16)
            .reshape(KO, 128, NI, SC)
            .transpose(1, 2, 0, 3)
        )
        for b in range(B)
    ]

    in_maps = []
    for c in range(N_CORES):
        b = c // (N_CORES // B)
        g = c % (N_CORES // B)
        es = g * E

        def pack_qk(w):
            rows = w[es : es + E]  # [E, D]
            blocks = [
                rows[h0 * DK : (h0 + 1) * DK][perm] for h0 in range(HPC)
            ]
            return _pack_pke(
                np.concatenate(blocks, axis=0).T.astype(np.float16)
            )

        def pack_bias(bvec):
            sl = bvec[es : es + E].reshape(HPC, DK)
            return np.ascontiguousarray(sl[:, perm].T)  # [DK, HPC]

        in_maps.append(
            {
                "xT": xT_b[b],
                "wqT": pack_qk(wq_w),
                "wkT": pack_qk(wk_w),
                "wvT": _pack_pke(wv_w[es : es + E].T.astype(np.float16)),
                "woT": np.ascontiguousarray(
                    wo_w[:, es : es + E].T.astype(np.float16)
                    .reshape(HPC, 128, D)
                    .transpose(1, 0, 2)
                ),
                "bq": pack_bias(wq_b),
                "bk": pack_bias(wk_b),
                "cc2": cc2,
                "sss": sss,
                "tri": tri,
                "ones": ones,
            }
        )

    trace = bool(os.environ.get("MHA_TRACE"))
    res = run_bass_kernel_spmd(
        nc, in_maps, list(range(N_CORES)), trace=trace
    )
    last_exec_time_ns = res.exec_time_ns
    last_results = res

    # host-side gather: sum partials per batch, add biases that commute
    # with attention (softmax rows sum to 1, so wv_b passes straight
    # through to the output projection)
    const_bias = wo_b + wo_w @ wv_b  # [D]
    out = np.empty((B, S, D), dtype=np.float32)
    gpb = N_CORES // B
    for b in range(B):
        acc = res.results[b * gpb]["out"].astype(np.float32)
        for c in range(b * gpb + 1, (b + 1) * gpb):
            acc += res.results[c]["out"].astype(np.float32)
        out[b] = acc + const_bias[None, :]
    return out


# revision 19
# speedup vs baseline: 1.3128x; 1.0123x over previous
"""Trainium2 Bass kernel for causal multi-head attention with RoPE.

Full-input contract: kernel(**inputs) takes the unsharded tensors and
returns the full [B, S, D] output. Internally the work is sharded over
8 NeuronCores: cores 0-3 compute batch 0, cores 4-7 batch 1; within a
batch group each core owns 4 of the 16 heads (tensor-parallel over
heads). Each core computes its partial output-projection contribution
[S, D]; the host sums the 4 partials per batch and adds the biases
that commute with attention (wo_b, and wv_b which passes through the
softmax untouched because attention weights sum to 1).

v3: all operands fp16 (half the DMA/SBUF of fp32r at the same PE
rate), every DRAM tensor pre-packed on the host so each DMA moves
contiguous 4KB-per-partition pieces (big descriptors - the v2 lesson:
rearranging in the DMA shatters loads into 1KB descriptors and the
input stream takes 25us). x/Q/K/V all stay resident in SBUF. V is
projected first, then per-head QK-projection + RoPE is software-
pipelined against the previous head's attention so exp latency hides
under projection matmuls. Causal masking is done with a column
prefill of -30000 into PSUM for the fully-masked columns plus a tiny
128x128 triangle multiply on the diagonal tile, which keeps the big
DVE ops off the QK->exp->PV critical path. Scores exp in 2-bank waves
(one ACT instruction per 1024 columns). The softmax denominator is a
fp16 DVE accumulation plus one ones-matmul per query chunk.
"""

import os
import sys

sys.path.insert(0, "/opt/trn_rl_repo")

import numpy as np

B = 2
S = 2048
D = 2048
H = 16
DK = 128
N_CORES = 8
HPC = 4          # heads per core
E = HPC * DK     # 512: per-core slice of the model dim
KO = D // 128    # contraction chunks for the projections
NJ = S // 128    # key blocks
SC = 512         # attention query chunk
NI = S // SC     # query chunks
ISQRT_DK = 1.0 / np.sqrt(DK)

_CACHE = {}

last_exec_time_ns = None
last_results = None


def _build_program():
    import concourse.mybir as mybir
    import concourse.tile as tile
    from concourse import bacc

    dt = mybir.dt
    F32 = dt.float32
    F16 = dt.float16
    AF = mybir.ActivationFunctionType

    nc = bacc.Bacc(None, target_bir_lowering=False, debug=True)

    # all tensors host-packed: partition dim first, contiguous free dims
    xT = nc.dram_tensor("xT", [128, NI, KO, SC], F16, kind="ExternalInput")
    wqT = nc.dram_tensor("wqT", [128, KO, E], F16, kind="ExternalInput")
    wkT = nc.dram_tensor("wkT", [128, KO, E], F16, kind="ExternalInput")
    wvT = nc.dram_tensor("wvT", [128, KO, E], F16, kind="ExternalInput")
    woT = nc.dram_tensor("woT", [128, HPC, D], F16, kind="ExternalInput")
    bq = nc.dram_tensor("bq", [DK, HPC], F32, kind="ExternalInput")
    bk = nc.dram_tensor("bk", [DK, HPC], F32, kind="ExternalInput")
    cc2 = nc.dram_tensor("cc2", [DK, S], F16, kind="ExternalInput")
    sss = nc.dram_tensor("sss", [DK, S], F16, kind="ExternalInput")
    tri = nc.dram_tensor("tri", [128, 128], F16, kind="ExternalInput")
    ones = nc.dram_tensor("ones", [128, 128], F16, kind="ExternalInput")
    out = nc.dram_tensor("out", [S, D], F16, kind="ExternalOutput")

    with tile.TileContext(nc) as tc:
        with (
            tc.tile_pool(name="const", bufs=1) as cpool,
            tc.tile_pool(name="res", bufs=1) as respool,
            tc.tile_pool(name="wqk", bufs=1) as wqkpool,
            tc.tile_pool(name="bwork", bufs=3) as p2pool,
            tc.tile_pool(name="bacc", bufs=2) as accpool,
            tc.tile_pool(name="bli", bufs=2) as lipool,
            tc.tile_pool(name="xres", bufs=1) as xpool,
        ):
            # ---- resident tiles ----
            x_sb = xpool.tile([128, NI, KO, SC], F16, name="x_sb")
            vt = respool.tile([128, NJ, E], F16, name="vt")
            qres = respool.tile([DK, HPC, S], F16, name="qres")
            kres = respool.tile([DK, HPC, S], F16, name="kres")
            aores = respool.tile([DK, HPC, S], F16, name="aores")
            wq_sb = wqkpool.tile([128, KO, E], F16, name="wq_sb")
            wk_sb = wqkpool.tile([128, KO, E], F16, name="wk_sb")

            # ---- input streams, need-ordered ----
            # wv split across the scalar/gpsimd queues, x chunks on sync;
            # everything is DRAM-contiguous per partition (128 descriptors)
            wv_ctx = tc.tile_pool(name="wv", bufs=1)
            wvpool = wv_ctx.__enter__()
            wv_sb = wvpool.tile([128, KO, E], F16, name="wv_sb")
            # si=0 in 4KB quarters + wv interleaved so the first V matmuls
            # start as early as possible; later si chunks stream behind
            for kq in range(4):
                ksl = slice(kq * 4, (kq + 1) * 4)
                (nc.sync if kq % 2 == 0 else nc.scalar).dma_start(
                    x_sb[:, 0, ksl, :], xT[:, 0, ksl, :]
                )
                nc.gpsimd.dma_start(
                    wv_sb[:, ksl, :], wvT[:, ksl, :]
                )
            xq = [nc.sync, nc.scalar, nc.gpsimd]
            for i in range(6):
                si, kh = 1 + i // 2, i % 2
                xq[i % 3].dma_start(
                    x_sb[:, si, kh * 8 : (kh + 1) * 8, :],
                    xT[:, si, kh * 8 : (kh + 1) * 8, :],
                )
            # constants
            bq_sb = cpool.tile([DK, HPC], F32, name="bq_sb")
            nc.gpsimd.dma_start(bq_sb[:], bq[:])
            bk_sb = cpool.tile([DK, HPC], F32, name="bk_sb")
            nc.gpsimd.dma_start(bk_sb[:], bk[:])
            cc2_sb = cpool.tile([DK, S], F16, name="cc2_sb")
            nc.gpsimd.dma_start(cc2_sb[:], cc2[:])
            sss_sb = cpool.tile([DK, S], F16, name="sss_sb")
            nc.gpsimd.dma_start(sss_sb[:], sss[:])
            tri_sb = cpool.tile([128, 128], F16, name="tri_sb")
            nc.gpsimd.dma_start(tri_sb[:], tri[:])
            ones_sb = cpool.tile([128, 128], F16, name="ones_sb")
            nc.gpsimd.dma_start(ones_sb[:], ones[:])
            # weights for Q/K projections (needed after V completes)
            for wsb, wdram in ((wk_sb, wkT), (wq_sb, wqT)):
                for g in range(4):
                    q = nc.scalar if g % 2 else nc.gpsimd
                    q.dma_start(
                        wsb[:, g * 4 : (g + 1) * 4, :],
                        wdram[:, g * 4 : (g + 1) * 4, :],
                    )

            # ---------- Phase V: value projection, vt resident ----------
            vps_ctx = tc.tile_pool(name="vps", bufs=4, space="PSUM")
            vpspool = vps_ctx.__enter__()
            for si in range(NI):
                if si == 0:
                    # k-outer so the PE consumes each x/wv quarter as it
                    # lands instead of stalling mid-chain on the first DMA
                    pvs = [
                        vpspool.tile([128, E], F32, tag="pv", name=f"pv0_{jj}")
                        for jj in range(4)
                    ]
                    for k in range(KO):
                        for jj in range(4):
                            jsl = slice(jj * 128, (jj + 1) * 128)
                            nc.tensor.matmul(
                                pvs[jj][:],
                                x_sb[:, si, k, jsl],
                                wv_sb[:, k, :],
                                start=(k == 0),
                                stop=(k == KO - 1),
                            )
                    for jj in range(4):
                        nc.vector.tensor_copy(vt[:, jj, :], pvs[jj][:])
                    continue
                for jj in range(4):
                    pv = vpspool.tile([128, E], F32, tag="pv")
                    jsl = slice(jj * 128, (jj + 1) * 128)
                    for k in range(KO):
                        nc.tensor.matmul(
                            pv[:],
                            x_sb[:, si, k, jsl],
                            wv_sb[:, k, :],
                            start=(k == 0),
                            stop=(k == KO - 1),
                        )
                    nc.vector.tensor_copy(vt[:, si * 4 + jj, :], pv[:])
            vps_ctx.__exit__(None, None, None)
            wv_ctx.__exit__(None, None, None)

            # ---------- interleaved QK projection + attention ----------
            qkps_ctx = tc.tile_pool(name="qkps", bufs=2, space="PSUM")
            pqpool = qkps_ctx.__enter__()
            bps_ctx = tc.tile_pool(name="bps", bufs=2, space="PSUM")
            ps2pool = bps_ctx.__enter__()
            bpo_ctx = tc.tile_pool(name="bpo", bufs=1, space="PSUM")
            popool = bpo_ctx.__enter__()
            bpl_ctx = tc.tile_pool(name="bpl", bufs=1, space="PSUM")
            plpool = bpl_ctx.__enter__()
            st_ctx = tc.tile_pool(name="stw", bufs=2)
            stpool = st_ctx.__enter__()
            sw_ctx = tc.tile_pool(name="sww", bufs=2)
            swpool = sw_ctx.__enter__()

            def qkproj_chunks(h):
                """8 emission chunks: (k, nsl0), (q, nsl0), (k, nsl1), ...
                RoPE per [128, 512] chunk: cross-partition swaps + cos-mul
                on DVE, sin-mul + add on gpsimd (small ops so neither queue
                blocks the attention pipeline for long)."""
                chunks = []
                for nsl in range(4):
                    for wsb, bsb, dst in (
                        (wk_sb, bk_sb, kres),
                        (wq_sb, bq_sb, qres),
                    ):
                        def emit(nsl=nsl, wsb=wsb, bsb=bsb, dst=dst):
                            sl = slice(nsl * SC, (nsl + 1) * SC)
                            pq = pqpool.tile([128, SC], F32, tag="pq")
                            for k in range(KO):
                                nc.tensor.matmul(
                                    pq[:],
                                    wsb[:, k, h * DK : (h + 1) * DK],
                                    x_sb[:, nsl, k, :],
                                    start=(k == 0),
                                    stop=(k == KO - 1),
                                )
                            st = stpool.tile([128, SC], F16, tag="st")
                            nc.scalar.activation(
                                st[:], pq[:], AF.Identity,
                                bias=bsb[:, h : h + 1],
                            )
                            sw = swpool.tile([128, SC], F16, tag="sw")
                            nc.vector.tensor_copy(sw[0:64, :], st[64:128, :])
                            nc.vector.tensor_copy(sw[64:128, :], st[0:64, :])
                            nc.gpsimd.tensor_mul(sw[:], sw[:], sss_sb[:, sl])
                            nc.vector.tensor_mul(
                                dst[:, h, sl], st[:], cc2_sb[:, sl]
                            )
                            nc.gpsimd.tensor_add(
                                dst[:, h, sl], dst[:, h, sl], sw[:]
                            )
                        chunks.append(emit)
                return chunks

            tails = []  # deferred [ones-matmul, recip, normalize] per chunk

            def attn_chunks(h):
                """4 emission chunks, one per query chunk ic."""
                chunks = []
                for ic in range(NI):
                    def emit(ic=ic):
                        njc = 4 * ic + 4
                        i0 = ic * SC
                        po = popool.tile([128, SC], F32, tag="po")
                        acc = accpool.tile([128, SC], F16, tag="acc")
                        pend = []

                        def flush(wave, ws2):
                            p2 = p2pool.tile([128, 2, SC], F16, tag="p2")
                            if wave[-1][1] >= 0:
                                # diagonal band: exp only the valid columns
                                for j, (jc, t, cs) in enumerate(wave):
                                    nc.scalar.activation(
                                        p2[:, j, cs:], ws2[:, j, cs:],
                                        AF.Exp, scale=float(ISQRT_DK),
                                    )
                            else:
                                nc.scalar.activation(
                                    p2[:, 0:2, :], ws2[:, 0:2, :], AF.Exp,
                                    scale=float(ISQRT_DK),
                                )
                            for j, (jc, t, cs) in enumerate(wave):
                                if t >= 0:
                                    # in-tile causal triangle
                                    nc.vector.tensor_mul(
                                        p2[:, j, cs : cs + 128],
                                        p2[:, j, cs : cs + 128],
                                        tri_sb[:],
                                    )
                                if jc == 0:
                                    nc.vector.tensor_copy(acc[:], p2[:, j, :])
                                else:
                                    nc.vector.tensor_add(
                                        acc[:, cs:], acc[:, cs:],
                                        p2[:, j, cs:],
                                    )
                            pend.append((p2, wave))

                        def drain_pv():
                            p2, wave = pend.pop(0)
                            for j, (jc, t, cs) in enumerate(wave):
                                nc.tensor.matmul(
                                    po[:, cs:],
                                    vt[:, jc, h * DK : (h + 1) * DK],
                                    p2[:, j, cs:],
                                    start=(jc == 0),
                                    stop=(jc == njc - 1),
                                )

                        wave, ws2 = [], None
                        first = True
                        for jc in range(njc):
                            t = jc - 4 * ic
                            cs = 128 * t if t >= 0 else 0
                            if not wave:
                                ws2 = ps2pool.tile([128, 2, SC], F32, tag="ps2")
                            nc.tensor.matmul(
                                ws2[:, len(wave), cs:],
                                kres[:, h, jc * 128 : (jc + 1) * 128],
                                qres[:, h, i0 + cs : i0 + SC],
                                start=True,
                                stop=True,
                            )
                            wave.append((jc, t, cs))
                            if len(wave) == 2:
                                flush(wave, ws2)
                                wave, ws2 = [], None
                                if first:
                                    # previous query chunk's softmax tail
                                    # lands here so its stragglers never
                                    # block this chunk's pipeline
                                    first = False
                                    if tails:
                                        tails.pop(0)()
                                if len(pend) > 1:
                                    drain_pv()
                        while pend:
                            drain_pv()

                        def tail(po=po, acc=acc, i0=i0):
                            pl = plpool.tile([128, SC], F32, tag="pl")
                            nc.tensor.matmul(
                                pl[:], ones_sb[:], acc[:],
                                start=True, stop=True,
                            )
                            li = lipool.tile([128, SC], F32, tag="li")
                            nc.vector.reciprocal_approx_fast(li[:], pl[:])
                            nc.vector.tensor_mul(
                                aores[:, h, i0 : i0 + SC], po[:], li[:]
                            )
                        tails.append(tail)
                    chunks.append(emit)
                return chunks

            # schedule: qkproj(0), then per head: attention(h) with
            # qkproj(h+1) chunks slotted after each query chunk
            for ch in qkproj_chunks(0):
                ch()
            for h in range(HPC - 1):
                nxt = qkproj_chunks(h + 1)
                at = attn_chunks(h)
                for ic in range(NI):
                    at[ic]()
                    for ch in nxt[2 * ic : 2 * ic + 2]:
                        ch()

            # projection scratch done; swap for output weights, prefetched
            # during the last head's attention
            sw_ctx.__exit__(None, None, None)
            st_ctx.__exit__(None, None, None)
            wo_ctx = tc.tile_pool(name="wo", bufs=1)
            wopool = wo_ctx.__enter__()
            wo_sb = wopool.tile([128, HPC, D], F16, name="wo_sb")
            for g in range(4):
                (nc.sync if g % 2 == 0 else nc.gpsimd).dma_start(
                    wo_sb[:, g, :], woT[:, g, :]
                )
            for ch in attn_chunks(HPC - 1):
                ch()
            while tails:
                tails.pop(0)()

            bpl_ctx.__exit__(None, None, None)
            bpo_ctx.__exit__(None, None, None)
            bps_ctx.__exit__(None, None, None)
            qkps_ctx.__exit__(None, None, None)

            # ---------- Phase C: output projection (partial sums) ----------
            with (
                tc.tile_pool(name="cob", bufs=2) as obpool,
                tc.tile_pool(name="cps", bufs=2, space="PSUM") as cpspool,
            ):
                for ii in range(S // 128):
                    isl = slice(ii * 128, (ii + 1) * 128)
                    ob = obpool.tile([128, D], F16, tag="ob")
                    for half in range(2):
                        pc = cpspool.tile([128, 2, 512], F32, tag="pc")
                        for ec in range(HPC):
                            for f2 in range(2):
                                fc = half * 2 + f2
                                nc.tensor.matmul(
                                    pc[:, f2, :],
                                    aores[:, ec, isl],
                                    wo_sb[:, ec, fc * 512 : (fc + 1) * 512],
                                    start=(ec == 0),
                                    stop=(ec == HPC - 1),
                                )
                        for f2 in range(2):
                            fc = half * 2 + f2
                            osl = slice(fc * 512, (fc + 1) * 512)
                            if f2 == 0:
                                nc.vector.tensor_copy(ob[:, osl], pc[:, f2, :])
                            else:
                                nc.scalar.activation(
                                    ob[:, osl], pc[:, f2, :], AF.Copy
                                )
                    (nc.sync if ii % 2 == 0 else nc.scalar).dma_start(
                        out[isl, :], ob[:]
                    )
            wo_ctx.__exit__(None, None, None)

    nc.compile()
    return nc


def _rope_tables():
    inv_freq = 1.0 / (10000.0 ** (np.arange(0, DK, 2, dtype=np.float64) / DK))
    pos = np.arange(S, dtype=np.float64)
    freqs = pos[:, None] * inv_freq[None, :]  # [S, DK/2]
    cos_t = np.cos(freqs).T.astype(np.float16)  # [64, S]
    sin_t = np.sin(freqs).T.astype(np.float16)
    cc2 = np.ascontiguousarray(np.concatenate([cos_t, cos_t], axis=0))
    sss = np.ascontiguousarray(np.concatenate([-sin_t, sin_t], axis=0))
    return cc2, sss


def _pack_pke(w16):
    """[D, E] -> [128, KO, E] with partition dim first, contiguous."""
    return np.ascontiguousarray(
        w16.reshape(KO, 128, E).transpose(1, 0, 2)
    )


def kernel(
    x, wq_w, wq_b, wk_w, wk_b, wv_w, wv_b, wo_w, wo_b
) -> np.ndarray:
    global last_exec_time_ns, last_results
    from concourse.bass_utils import run_bass_kernel_spmd

    if "nc" not in _CACHE:
        _CACHE["nc"] = _build_program()
    nc = _CACHE["nc"]

    x = np.asarray(x, dtype=np.float32)
    wq_w = np.asarray(wq_w, dtype=np.float32)
    wk_w = np.asarray(wk_w, dtype=np.float32)
    wv_w = np.asarray(wv_w, dtype=np.float32)
    wo_w = np.asarray(wo_w, dtype=np.float32)
    wq_b = np.asarray(wq_b, dtype=np.float32)
    wk_b = np.asarray(wk_b, dtype=np.float32)
    wv_b = np.asarray(wv_b, dtype=np.float32)
    wo_b = np.asarray(wo_b, dtype=np.float32)

    cc2, sss = _rope_tables()
    r_idx = np.arange(128)[:, None]
    c_idx = np.arange(128)[None, :]
    tri = np.ascontiguousarray((r_idx <= c_idx).astype(np.float16))
    ones = np.ones((128, 128), dtype=np.float16)
    # within each head, pack d-rows as [even dims; odd dims]
    perm = np.concatenate([np.arange(0, DK, 2), np.arange(1, DK, 2)])

    # x: [S, D] -> xT [D, S] -> [128, NI(si), KO(k), SC] contiguous
    xT_b = [
        np.ascontiguousarray(
            x[b].T.astype(np.float16)
            .reshape(KO, 128, NI, SC)
            .transpose(1, 2, 0, 3)
        )
        for b in range(B)
    ]

    in_maps = []
    for c in range(N_CORES):
        b = c // (N_CORES // B)
        g = c % (N_CORES // B)
        es = g * E

        def pack_qk(w):
            rows = w[es : es + E]  # [E, D]
            blocks = [
                rows[h0 * DK : (h0 + 1) * DK][perm] for h0 in range(HPC)
            ]
            return _pack_pke(
                np.concatenate(blocks, axis=0).T.astype(np.float16)
            )

        def pack_bias(bvec):
            sl = bvec[es : es + E].reshape(HPC, DK)
            return np.ascontiguousarray(sl[:, perm].T)  # [DK, HPC]

        in_maps.append(
            {
                "xT": xT_b[b],
                "wqT": pack_qk(wq_w),
                "wkT": pack_qk(wk_w),
                "wvT": _pack_pke(wv_w[es : es + E].T.astype(np.float16)),
                "woT": np.ascontiguousarray(
                    wo_w[:, es : es + E].T.astype(np.float16)
                    .reshape(HPC, 128, D)
                    .transpose(1, 0, 2)
                ),
                "bq": pack_bias(wq_b),
                "bk": pack_bias(wk_b),
                "cc2": cc2,
                "sss": sss,
                "tri": tri,
                "ones": ones,
            }
        )

    trace = bool(os.environ.get("MHA_TRACE"))
    res = run_bass_kernel_spmd(
        nc, in_maps, list(range(N_CORES)), trace=trace
    )
    last_exec_time_ns = res.exec_time_ns
    last_results = res

    # host-side gather: sum partials per batch, add biases that commute
    # with attention (softmax rows sum to 1, so wv_b passes straight
    # through to the output projection)
    const_bias = wo_b + wo_w @ wv_b  # [D]
    out = np.empty((B, S, D), dtype=np.float32)
    gpb = N_CORES // B
    for b in range(B):
        acc = res.results[b * gpb]["out"].astype(np.float32)
        for c in range(b * gpb + 1, (b + 1) * gpb):
            acc += res.results[c]["out"].astype(np.float32)
        out[b] = acc + const_bias[None, :]
    return out


# revision 21
# speedup vs baseline: 1.3165x; 1.0028x over previous
"""Trainium2 Bass kernel for causal multi-head attention with RoPE.

Full-input contract: kernel(**inputs) takes the unsharded tensors and
returns the full [B, S, D] output. Internally the work is sharded over
8 NeuronCores: cores 0-3 compute batch 0, cores 4-7 batch 1; within a
batch group each core owns 4 of the 16 heads (tensor-parallel over
heads). Each core computes its partial output-projection contribution
[S, D]; the host sums the 4 partials per batch and adds the biases
that commute with attention (wo_b, and wv_b which passes through the
softmax untouched because attention weights sum to 1).

v3: all operands fp16 (half the DMA/SBUF of fp32r at the same PE
rate), every DRAM tensor pre-packed on the host so each DMA moves
contiguous 4KB-per-partition pieces (big descriptors - the v2 lesson:
rearranging in the DMA shatters loads into 1KB descriptors and the
input stream takes 25us). x/Q/K/V all stay resident in SBUF. V is
projected first, then per-head QK-projection + RoPE is software-
pipelined against the previous head's attention so exp latency hides
under projection matmuls. Causal masking is done with a column
prefill of -30000 into PSUM for the fully-masked columns plus a tiny
128x128 triangle multiply on the diagonal tile, which keeps the big
DVE ops off the QK->exp->PV critical path. Scores exp in 2-bank waves
(one ACT instruction per 1024 columns). The softmax denominator is a
fp16 DVE accumulation plus one ones-matmul per query chunk.
"""

import os
import sys

sys.path.insert(0, "/opt/trn_rl_repo")

import numpy as np

B = 2
S = 2048
D = 2048
H = 16
DK = 128
N_CORES = 8
HPC = 4          # heads per core
E = HPC * DK     # 512: per-core slice of the model dim
KO = D // 128    # contraction chunks for the projections
NJ = S // 128    # key blocks
SC = 512         # attention query chunk
NI = S // SC     # query chunks
ISQRT_DK = 1.0 / np.sqrt(DK)

_CACHE = {}

last_exec_time_ns = None
last_results = None


def _build_program():
    import concourse.mybir as mybir
    import concourse.tile as tile
    from concourse import bacc

    dt = mybir.dt
    F32 = dt.float32
    F16 = dt.float16
    AF = mybir.ActivationFunctionType

    nc = bacc.Bacc(None, target_bir_lowering=False, debug=True)

    # all tensors host-packed: partition dim first, contiguous free dims
    xT = nc.dram_tensor("xT", [128, NI, KO, SC], F16, kind="ExternalInput")
    wqT = nc.dram_tensor("wqT", [128, KO, E], F16, kind="ExternalInput")
    wkT = nc.dram_tensor("wkT", [128, KO, E], F16, kind="ExternalInput")
    wvT = nc.dram_tensor("wvT", [128, KO, E], F16, kind="ExternalInput")
    woT = nc.dram_tensor("woT", [128, HPC, D], F16, kind="ExternalInput")
    bq = nc.dram_tensor("bq", [DK, HPC], F32, kind="ExternalInput")
    bk = nc.dram_tensor("bk", [DK, HPC], F32, kind="ExternalInput")
    cc2 = nc.dram_tensor("cc2", [DK, S], F16, kind="ExternalInput")
    sss = nc.dram_tensor("sss", [DK, S], F16, kind="ExternalInput")
    tri = nc.dram_tensor("tri", [128, 128], F16, kind="ExternalInput")
    ones = nc.dram_tensor("ones", [128, 128], F16, kind="ExternalInput")
    out = nc.dram_tensor("out", [S, D], F16, kind="ExternalOutput")

    with tile.TileContext(nc) as tc:
        with (
            tc.tile_pool(name="const", bufs=1) as cpool,
            tc.tile_pool(name="res", bufs=1) as respool,
            tc.tile_pool(name="wqk", bufs=1) as wqkpool,
            tc.tile_pool(name="bwork", bufs=4) as p2pool,
            tc.tile_pool(name="bacc", bufs=2) as accpool,
            tc.tile_pool(name="bli", bufs=2) as lipool,
            tc.tile_pool(name="xres", bufs=1) as xpool,
        ):
            # ---- resident tiles ----
            x_sb = xpool.tile([128, NI, KO, SC], F16, name="x_sb")
            vt = respool.tile([128, NJ, E], F16, name="vt")
            qres = respool.tile([DK, HPC, S], F16, name="qres")
            kres = respool.tile([DK, HPC, S], F16, name="kres")
            aores = respool.tile([DK, HPC, S], F16, name="aores")
            wq_sb = wqkpool.tile([128, KO, E], F16, name="wq_sb")
            wk_sb = wqkpool.tile([128, KO, E], F16, name="wk_sb")

            # ---- input streams, need-ordered ----
            # wv split across the scalar/gpsimd queues, x chunks on sync;
            # everything is DRAM-contiguous per partition (128 descriptors)
            wv_ctx = tc.tile_pool(name="wv", bufs=1)
            wvpool = wv_ctx.__enter__()
            wv_sb = wvpool.tile([128, KO, E], F16, name="wv_sb")
            # si=0 and wv in 4KB quarters, strictly need-ordered across the
            # sync/scalar rings (the 16 DMA engines drain rings fairly, so
            # the first V chain's operands must be at the ring heads with
            # nothing competing); all later traffic goes behind on gpsimd
            xwq = [nc.sync, nc.scalar]
            for kq in range(4):
                ksl = slice(kq * 4, (kq + 1) * 4)
                xwq[kq % 2].dma_start(x_sb[:, 0, ksl, :], xT[:, 0, ksl, :])
                xwq[1 - kq % 2].dma_start(wv_sb[:, ksl, :], wvT[:, ksl, :])
            xq = [nc.sync, nc.scalar, nc.gpsimd]
            for i in range(6):
                si, kh = 1 + i // 2, i % 2
                xq[i % 3].dma_start(
                    x_sb[:, si, kh * 8 : (kh + 1) * 8, :],
                    xT[:, si, kh * 8 : (kh + 1) * 8, :],
                )
            # constants
            bq_sb = cpool.tile([DK, HPC], F32, name="bq_sb")
            nc.gpsimd.dma_start(bq_sb[:], bq[:])
            bk_sb = cpool.tile([DK, HPC], F32, name="bk_sb")
            nc.gpsimd.dma_start(bk_sb[:], bk[:])
            cc2_sb = cpool.tile([DK, S], F16, name="cc2_sb")
            nc.gpsimd.dma_start(cc2_sb[:], cc2[:])
            sss_sb = cpool.tile([DK, S], F16, name="sss_sb")
            nc.gpsimd.dma_start(sss_sb[:], sss[:])
            tri_sb = cpool.tile([128, 128], F16, name="tri_sb")
            nc.gpsimd.dma_start(tri_sb[:], tri[:])
            ones_sb = cpool.tile([128, 128], F16, name="ones_sb")
            nc.gpsimd.dma_start(ones_sb[:], ones[:])
            # weights for Q/K projections (needed after V completes)
            for wsb, wdram in ((wk_sb, wkT), (wq_sb, wqT)):
                for g in range(4):
                    q = nc.scalar if g % 2 else nc.gpsimd
                    q.dma_start(
                        wsb[:, g * 4 : (g + 1) * 4, :],
                        wdram[:, g * 4 : (g + 1) * 4, :],
                    )

            # ---------- Phase V: value projection, vt resident ----------
            vps_ctx = tc.tile_pool(name="vps", bufs=4, space="PSUM")
            vpspool = vps_ctx.__enter__()
            for si in range(NI):
                if si == 0:
                    # k-outer so the PE consumes each x/wv quarter as it
                    # lands instead of stalling mid-chain on the first DMA
                    pvs = [
                        vpspool.tile([128, E], F32, tag="pv", name=f"pv0_{jj}")
                        for jj in range(4)
                    ]
                    for k in range(KO):
                        for jj in range(4):
                            jsl = slice(jj * 128, (jj + 1) * 128)
                            nc.tensor.matmul(
                                pvs[jj][:],
                                x_sb[:, si, k, jsl],
                                wv_sb[:, k, :],
                                start=(k == 0),
                                stop=(k == KO - 1),
                            )
                    for jj in range(4):
                        nc.vector.tensor_copy(vt[:, jj, :], pvs[jj][:])
                    continue
                for jj in range(4):
                    pv = vpspool.tile([128, E], F32, tag="pv")
                    jsl = slice(jj * 128, (jj + 1) * 128)
                    for k in range(KO):
                        nc.tensor.matmul(
                            pv[:],
                            x_sb[:, si, k, jsl],
                            wv_sb[:, k, :],
                            start=(k == 0),
                            stop=(k == KO - 1),
                        )
                    nc.vector.tensor_copy(vt[:, si * 4 + jj, :], pv[:])
            vps_ctx.__exit__(None, None, None)
            wv_ctx.__exit__(None, None, None)

            # ---------- interleaved QK projection + attention ----------
            qkps_ctx = tc.tile_pool(name="qkps", bufs=2, space="PSUM")
            pqpool = qkps_ctx.__enter__()
            bps_ctx = tc.tile_pool(name="bps", bufs=2, space="PSUM")
            ps2pool = bps_ctx.__enter__()
            bpo_ctx = tc.tile_pool(name="bpo", bufs=1, space="PSUM")
            popool = bpo_ctx.__enter__()
            bpl_ctx = tc.tile_pool(name="bpl", bufs=1, space="PSUM")
            plpool = bpl_ctx.__enter__()
            st_ctx = tc.tile_pool(name="stw", bufs=4)
            stpool = st_ctx.__enter__()
            sw_ctx = tc.tile_pool(name="sww", bufs=4)
            swpool = sw_ctx.__enter__()

            def qkproj_chunks(h):
                """8 emission chunks: (k, nsl0), (q, nsl0), (k, nsl1), ...
                RoPE per [128, 512] chunk: cross-partition swaps + cos-mul
                on DVE, sin-mul + add on gpsimd (small ops so neither queue
                blocks the attention pipeline for long)."""
                chunks = []
                for nsl in range(4):
                    for wsb, bsb, dst in (
                        (wk_sb, bk_sb, kres),
                        (wq_sb, bq_sb, qres),
                    ):
                        def emit(nsl=nsl, wsb=wsb, bsb=bsb, dst=dst):
                            sl = slice(nsl * SC, (nsl + 1) * SC)
                            pq = pqpool.tile([128, SC], F32, tag="pq")
                            for k in range(KO):
                                nc.tensor.matmul(
                                    pq[:],
                                    wsb[:, k, h * DK : (h + 1) * DK],
                                    x_sb[:, nsl, k, :],
                                    start=(k == 0),
                                    stop=(k == KO - 1),
                                )
                            st = stpool.tile([128, SC], F16, tag="st")
                            nc.scalar.activation(
                                st[:], pq[:], AF.Identity,
                                bias=bsb[:, h : h + 1],
                            )
                            sw = swpool.tile([128, SC], F16, tag="sw")
                            nc.vector.tensor_copy(sw[0:64, :], st[64:128, :])
                            nc.vector.tensor_copy(sw[64:128, :], st[0:64, :])
                            nc.gpsimd.tensor_mul(sw[:], sw[:], sss_sb[:, sl])
                            nc.vector.tensor_mul(
                                dst[:, h, sl], st[:], cc2_sb[:, sl]
                            )
                            nc.gpsimd.tensor_add(
                                dst[:, h, sl], dst[:, h, sl], sw[:]
                            )
                        chunks.append(emit)
                return chunks

            tails = []  # deferred [ones-matmul, recip, normalize] per chunk

            def attn_chunks(h):
                """4 emission chunks, one per query chunk ic."""
                chunks = []
                for ic in range(NI):
                    def emit(ic=ic):
                        njc = 4 * ic + 4
                        i0 = ic * SC
                        po = popool.tile([128, SC], F32, tag="po")
                        acc = accpool.tile([128, SC], F16, tag="acc")
                        pend = []

                        def flush(wave, ws2):
                            p2 = p2pool.tile([128, 2, SC], F16, tag="p2")
                            if wave[-1][1] >= 0:
                                # diagonal band: exp only the valid columns
                                for j, (jc, t, cs) in enumerate(wave):
                                    nc.scalar.activation(
                                        p2[:, j, cs:], ws2[:, j, cs:],
                                        AF.Exp, scale=float(ISQRT_DK),
                                    )
                            else:
                                nc.scalar.activation(
                                    p2[:, 0:2, :], ws2[:, 0:2, :], AF.Exp,
                                    scale=float(ISQRT_DK),
                                )
                            for j, (jc, t, cs) in enumerate(wave):
                                if t >= 0:
                                    # in-tile causal triangle
                                    nc.vector.tensor_mul(
                                        p2[:, j, cs : cs + 128],
                                        p2[:, j, cs : cs + 128],
                                        tri_sb[:],
                                    )
                                if jc == 0:
                                    nc.vector.tensor_copy(acc[:], p2[:, j, :])
                                else:
                                    nc.vector.tensor_add(
                                        acc[:, cs:], acc[:, cs:],
                                        p2[:, j, cs:],
                                    )
                            pend.append((p2, wave))

                        def drain_pv():
                            p2, wave = pend.pop(0)
                            for j, (jc, t, cs) in enumerate(wave):
                                nc.tensor.matmul(
                                    po[:, cs:],
                                    vt[:, jc, h * DK : (h + 1) * DK],
                                    p2[:, j, cs:],
                                    start=(jc == 0),
                                    stop=(jc == njc - 1),
                                )

                        wave, ws2 = [], None
                        first = True
                        for jc in range(njc):
                            t = jc - 4 * ic
                            cs = 128 * t if t >= 0 else 0
                            if not wave:
                                ws2 = ps2pool.tile([128, 2, SC], F32, tag="ps2")
                            nc.tensor.matmul(
                                ws2[:, len(wave), cs:],
                                kres[:, h, jc * 128 : (jc + 1) * 128],
                                qres[:, h, i0 + cs : i0 + SC],
                                start=True,
                                stop=True,
                            )
                            wave.append((jc, t, cs))
                            if len(wave) == 2:
                                flush(wave, ws2)
                                wave, ws2 = [], None
                                if first:
                                    # previous query chunk's softmax tail
                                    # lands here so its stragglers never
                                    # block this chunk's pipeline
                                    first = False
                                    if tails:
                                        tails.pop(0)()
                                if len(pend) > 1:
                                    drain_pv()
                        while pend:
                            drain_pv()

                        def tail(po=po, acc=acc, i0=i0):
                            pl = plpool.tile([128, SC], F32, tag="pl")
                            nc.tensor.matmul(
                                pl[:], ones_sb[:], acc[:],
                                start=True, stop=True,
                            )
                            li = lipool.tile([128, SC], F32, tag="li")
                            nc.vector.reciprocal_approx_fast(li[:], pl[:])
                            nc.vector.tensor_mul(
                                aores[:, h, i0 : i0 + SC], po[:], li[:]
                            )
                        tails.append(tail)
                    chunks.append(emit)
                return chunks

            # schedule: qkproj(0), then per head: attention(h) with
            # qkproj(h+1) chunks slotted after each query chunk
            for ch in qkproj_chunks(0):
                ch()
            for h in range(HPC - 1):
                nxt = qkproj_chunks(h + 1)
                at = attn_chunks(h)
                for ic in range(NI):
                    at[ic]()
                    for ch in nxt[2 * ic : 2 * ic + 2]:
                        ch()

            # projection scratch done; swap for output weights, prefetched
            # during the last head's attention
            sw_ctx.__exit__(None, None, None)
            st_ctx.__exit__(None, None, None)
            wo_ctx = tc.tile_pool(name="wo", bufs=1)
            wopool = wo_ctx.__enter__()
            wo_sb = wopool.tile([128, HPC, D], F16, name="wo_sb")
            for g in range(4):
                (nc.sync if g % 2 == 0 else nc.gpsimd).dma_start(
                    wo_sb[:, g, :], woT[:, g, :]
                )
            for ch in attn_chunks(HPC - 1):
                ch()
            while tails:
                tails.pop(0)()

            bpl_ctx.__exit__(None, None, None)
            bpo_ctx.__exit__(None, None, None)
            bps_ctx.__exit__(None, None, None)
            qkps_ctx.__exit__(None, None, None)

            # ---------- Phase C: output projection (partial sums) ----------
            with (
                tc.tile_pool(name="cob", bufs=2) as obpool,
                tc.tile_pool(name="cps", bufs=2, space="PSUM") as cpspool,
            ):
                for ii in range(S // 128):
                    isl = slice(ii * 128, (ii + 1) * 128)
                    ob = obpool.tile([128, D], F16, tag="ob")
                    for half in range(2):
                        pc = cpspool.tile([128, 2, 512], F32, tag="pc")
                        for ec in range(HPC):
                            for f2 in range(2):
                                fc = half * 2 + f2
                                nc.tensor.matmul(
                                    pc[:, f2, :],
                                    aores[:, ec, isl],
                                    wo_sb[:, ec, fc * 512 : (fc + 1) * 512],
                                    start=(ec == 0),
                                    stop=(ec == HPC - 1),
                                )
                        for f2 in range(2):
                            fc = half * 2 + f2
                            osl = slice(fc * 512, (fc + 1) * 512)
                            if f2 == 0:
                                nc.vector.tensor_copy(ob[:, osl], pc[:, f2, :])
                            else:
                                nc.scalar.activation(
                                    ob[:, osl], pc[:, f2, :], AF.Copy
                                )
                    (nc.sync if ii % 2 == 0 else nc.scalar).dma_start(
                        out[isl, :], ob[:]
                    )
            wo_ctx.__exit__(None, None, None)

    nc.compile()
    return nc


def _rope_tables():
    inv_freq = 1.0 / (10000.0 ** (np.arange(0, DK, 2, dtype=np.float64) / DK))
    pos = np.arange(S, dtype=np.float64)
    freqs = pos[:, None] * inv_freq[None, :]  # [S, DK/2]
    cos_t = np.cos(freqs).T.astype(np.float16)  # [64, S]
    sin_t = np.sin(freqs).T.astype(np.float16)
    cc2 = np.ascontiguousarray(np.concatenate([cos_t, cos_t], axis=0))
    sss = np.ascontiguousarray(np.concatenate([-sin_t, sin_t], axis=0))
    return cc2, sss


def _pack_pke(w16):
    """[D, E] -> [128, KO, E] with partition dim first, contiguous."""
    return np.ascontiguousarray(
        w16.reshape(KO, 128, E).transpose(1, 0, 2)
    )


def kernel(
    x, wq_w, wq_b, wk_w, wk_b, wv_w, wv_b, wo_w, wo_b
) -> np.ndarray:
    global last_exec_time_ns, last_results
    from concourse.bass_utils import run_bass_kernel_spmd

    if "nc" not in _CACHE:
        _CACHE["nc"] = _build_program()
    nc = _CACHE["nc"]

    x = np.asarray(x, dtype=np.float32)
    wq_w = np.asarray(wq_w, dtype=np.float32)
    wk_w = np.asarray(wk_w, dtype=np.float32)
    wv_w = np.asarray(wv_w, dtype=np.float32)
    wo_w = np.asarray(wo_w, dtype=np.float32)
    wq_b = np.asarray(wq_b, dtype=np.float32)
    wk_b = np.asarray(wk_b, dtype=np.float32)
    wv_b = np.asarray(wv_b, dtype=np.float32)
    wo_b = np.asarray(wo_b, dtype=np.float32)

    cc2, sss = _rope_tables()
    r_idx = np.arange(128)[:, None]
    c_idx = np.arange(128)[None, :]
    tri = np.ascontiguousarray((r_idx <= c_idx).astype(np.float16))
    ones = np.ones((128, 128), dtype=np.float16)
    # within each head, pack d-rows as [even dims; odd dims]
    perm = np.concatenate([np.arange(0, DK, 2), np.arange(1, DK, 2)])

    # x: [S, D] -> xT [D, S] -> [128, NI(si), KO(k), SC] contiguous
    xT_b = [
        np.ascontiguousarray(
            x[b].T.astype(np.float16)
            .reshape(KO, 128, NI, SC)
            .transpose(1, 2, 0, 3)
        )
        for b in range(B)
    ]

    in_maps = []
    for c in range(N_CORES):
        b = c // (N_CORES // B)
        g = c % (N_CORES // B)
        es = g * E

        def pack_qk(w):
            rows = w[es : es + E]  # [E, D]
            blocks = [
                rows[h0 * DK : (h0 + 1) * DK][perm] for h0 in range(HPC)
            ]
            return _pack_pke(
                np.concatenate(blocks, axis=0).T.astype(np.float16)
            )

        def pack_bias(bvec):
            sl = bvec[es : es + E].reshape(HPC, DK)
            return np.ascontiguousarray(sl[:, perm].T)  # [DK, HPC]

        in_maps.append(
            {
                "xT": xT_b[b],
                "wqT": pack_qk(wq_w),
                "wkT": pack_qk(wk_w),
                "wvT": _pack_pke(wv_w[es : es + E].T.astype(np.float16)),
                "woT": np.ascontiguousarray(
                    wo_w[:, es : es + E].T.astype(np.float16)
                    .reshape(HPC, 128, D)
                    .transpose(1, 0, 2)
                ),
                "bq": pack_bias(wq_b),
                "bk": pack_bias(wk_b),
                "cc2": cc2,
                "sss": sss,
                "tri": tri,
                "ones": ones,
            }
        )

    trace = bool(os.environ.get("MHA_TRACE"))
    res = run_bass_kernel_spmd(
        nc, in_maps, list(range(N_CORES)), trace=trace
    )
    last_exec_time_ns = res.exec_time_ns
    last_results = res

    # host-side gather: sum partials per batch, add biases that commute
    # with attention (softmax rows sum to 1, so wv_b passes straight
    # through to the output projection)
    const_bias = wo_b + wo_w @ wv_b  # [D]
    out = np.empty((B, S, D), dtype=np.float32)
    gpb = N_CORES // B
    for b in range(B):
        acc = res.results[b * gpb]["out"].astype(np.float32)
        for c in range(b * gpb + 1, (b + 1) * gpb):
            acc += res.results[c]["out"].astype(np.float32)
        out[b] = acc + const_bias[None, :]
    return out


# revision 25
# speedup vs baseline: 1.3978x; 1.0618x over previous
"""Trainium2 Bass kernel for causal multi-head attention with RoPE.

Full-input contract: kernel(**inputs) takes the unsharded tensors and
returns the full [B, S, D] output. Internally the work is sharded over
8 NeuronCores: cores 0-3 compute batch 0, cores 4-7 batch 1; within a
batch group each core owns 4 of the 16 heads (tensor-parallel over
heads). Each core computes its partial output-projection contribution
[S, D]; the host sums the 4 partials per batch and adds the biases
that commute with attention (wo_b, and wv_b which passes through the
softmax untouched because attention weights sum to 1).

v3: all operands fp16 (half the DMA/SBUF of fp32r at the same PE
rate), every DRAM tensor pre-packed on the host so each DMA moves
contiguous 4KB-per-partition pieces (big descriptors - the v2 lesson:
rearranging in the DMA shatters loads into 1KB descriptors and the
input stream takes 25us). x/Q/K/V all stay resident in SBUF. V is
projected first, then per-head QK-projection + RoPE is software-
pipelined against the previous head's attention so exp latency hides
under projection matmuls. Causal masking is done with a column
prefill of -30000 into PSUM for the fully-masked columns plus a tiny
128x128 triangle multiply on the diagonal tile, which keeps the big
DVE ops off the QK->exp->PV critical path. Scores exp in 2-bank waves
(one ACT instruction per 1024 columns). The softmax denominator is a
fp16 DVE accumulation plus one ones-matmul per query chunk.
"""

import os
import sys

sys.path.insert(0, "/opt/trn_rl_repo")

import numpy as np

B = 2
S = 2048
D = 2048
H = 16
DK = 128
N_CORES = 8
HPC = 4          # heads per core
E = HPC * DK     # 512: per-core slice of the model dim
KO = D // 128    # contraction chunks for the projections
NJ = S // 128    # key blocks
SC = 512         # attention query chunk
NI = S // SC     # query chunks
ISQRT_DK = 1.0 / np.sqrt(DK)

_CACHE = {}

last_exec_time_ns = None
last_results = None


def _build_program():
    import concourse.mybir as mybir
    import concourse.tile as tile
    from concourse import bacc

    dt = mybir.dt
    F32 = dt.float32
    F16 = dt.float16
    AF = mybir.ActivationFunctionType

    nc = bacc.Bacc(None, target_bir_lowering=False, debug=True)

    # all tensors host-packed: partition dim first, contiguous free dims
    xT = nc.dram_tensor("xT", [128, NI, KO, SC], F16, kind="ExternalInput")
    wqT = nc.dram_tensor("wqT", [128, KO, E], F16, kind="ExternalInput")
    wkT = nc.dram_tensor("wkT", [128, KO, E], F16, kind="ExternalInput")
    wvT = nc.dram_tensor("wvT", [128, KO, E], F16, kind="ExternalInput")
    woT = nc.dram_tensor("woT", [128, HPC, D], F16, kind="ExternalInput")
    bq = nc.dram_tensor("bq", [DK, HPC], F32, kind="ExternalInput")
    bk = nc.dram_tensor("bk", [DK, HPC], F32, kind="ExternalInput")
    cc2 = nc.dram_tensor("cc2", [DK, S], F16, kind="ExternalInput")
    sss = nc.dram_tensor("sss", [DK, S], F16, kind="ExternalInput")
    tri = nc.dram_tensor("tri", [128, 128], F16, kind="ExternalInput")
    ones = nc.dram_tensor("ones", [128, 128], F16, kind="ExternalInput")
    out = nc.dram_tensor("out", [S, D], F16, kind="ExternalOutput")

    with tile.TileContext(nc) as tc:
        with (
            tc.tile_pool(name="const", bufs=1) as cpool,
            tc.tile_pool(name="res", bufs=1) as respool,
            tc.tile_pool(name="wqk", bufs=1) as wqkpool,
            tc.tile_pool(name="bwork", bufs=4) as p2pool,
            tc.tile_pool(name="bacc", bufs=2) as accpool,
            tc.tile_pool(name="bli", bufs=2) as lipool,
            tc.tile_pool(name="xres", bufs=1) as xpool,
        ):
            # ---- resident tiles ----
            x_sb = xpool.tile([128, NI, KO, SC], F16, name="x_sb")
            vt = respool.tile([128, NJ, E], F16, name="vt")
            qres = respool.tile([DK, HPC, S], F16, name="qres")
            kres = respool.tile([DK, HPC, S], F16, name="kres")
            aores = respool.tile([DK, HPC, S], F16, name="aores")
            wq_sb = wqkpool.tile([128, KO, E], F16, name="wq_sb")
            wk_sb = wqkpool.tile([128, KO, E], F16, name="wk_sb")

            # ---- input streams, need-ordered ----
            # wv split across the scalar/gpsimd queues, x chunks on sync;
            # everything is DRAM-contiguous per partition (128 descriptors)
            wv_ctx = tc.tile_pool(name="wv", bufs=1)
            wvpool = wv_ctx.__enter__()
            wv_sb = wvpool.tile([128, KO, E], F16, name="wv_sb")
            # si=0 and wv in 4KB quarters, strictly need-ordered across the
            # sync/scalar rings (the 16 DMA engines drain rings fairly, so
            # the first V chain's operands must be at the ring heads with
            # nothing competing); all later traffic goes behind on gpsimd
            pieces = []
            for kq in range(4):
                ksl = slice(kq * 4, (kq + 1) * 4)
                pieces.append((x_sb[:, 0, ksl, :], xT[:, 0, ksl, :]))
                pieces.append((wv_sb[:, ksl, :], wvT[:, ksl, :]))
            for si in range(1, NI):
                for kq in range(4):
                    ksl = slice(kq * 4, (kq + 1) * 4)
                    pieces.append((x_sb[:, si, ksl, :], xT[:, si, ksl, :]))
            xq = [nc.sync, nc.scalar, nc.gpsimd]
            for i, (dst, src) in enumerate(pieces):
                xq[i % 3].dma_start(dst, src)
            # constants
            bq_sb = cpool.tile([DK, HPC], F32, name="bq_sb")
            nc.gpsimd.dma_start(bq_sb[:], bq[:])
            bk_sb = cpool.tile([DK, HPC], F32, name="bk_sb")
            nc.gpsimd.dma_start(bk_sb[:], bk[:])
            cc2_sb = cpool.tile([DK, S], F16, name="cc2_sb")
            nc.gpsimd.dma_start(cc2_sb[:], cc2[:])
            sss_sb = cpool.tile([DK, S], F16, name="sss_sb")
            nc.gpsimd.dma_start(sss_sb[:], sss[:])
            tri_sb = cpool.tile([128, 128], F16, name="tri_sb")
            nc.gpsimd.dma_start(tri_sb[:], tri[:])
            ones_sb = cpool.tile([128, 128], F16, name="ones_sb")
            nc.gpsimd.dma_start(ones_sb[:], ones[:])
            # weights for Q/K projections (needed after V completes)
            wp = []
            for wsb, wdram in ((wk_sb, wkT), (wq_sb, wqT)):
                for g in range(4):
                    wp.append(
                        (
                            wsb[:, g * 4 : (g + 1) * 4, :],
                            wdram[:, g * 4 : (g + 1) * 4, :],
                        )
                    )
            for i, (dst, src) in enumerate(wp):
                xq[i % 3].dma_start(dst, src)

            # ---------- Phase V: value projection, vt resident ----------
            vps_ctx = tc.tile_pool(name="vps", bufs=4, space="PSUM")
            vpspool = vps_ctx.__enter__()
            for si in range(NI):
                if si == 0:
                    # k-outer so the PE consumes each x/wv quarter as it
                    # lands instead of stalling mid-chain on the first DMA
                    pvs = [
                        vpspool.tile([128, E], F32, tag="pv", name=f"pv0_{jj}")
                        for jj in range(4)
                    ]
                    for k in range(KO):
                        for jj in range(4):
                            jsl = slice(jj * 128, (jj + 1) * 128)
                            nc.tensor.matmul(
                                pvs[jj][:],
                                x_sb[:, si, k, jsl],
                                wv_sb[:, k, :],
                                start=(k == 0),
                                stop=(k == KO - 1),
                            )
                    for jj in range(4):
                        nc.vector.tensor_copy(vt[:, jj, :], pvs[jj][:])
                    continue
                for jj in range(4):
                    pv = vpspool.tile([128, E], F32, tag="pv")
                    jsl = slice(jj * 128, (jj + 1) * 128)
                    for k in range(KO):
                        nc.tensor.matmul(
                            pv[:],
                            x_sb[:, si, k, jsl],
                            wv_sb[:, k, :],
                            start=(k == 0),
                            stop=(k == KO - 1),
                        )
                    nc.vector.tensor_copy(vt[:, si * 4 + jj, :], pv[:])
            vps_ctx.__exit__(None, None, None)
            wv_ctx.__exit__(None, None, None)

            # ---------- interleaved QK projection + attention ----------
            qkps_ctx = tc.tile_pool(name="qkps", bufs=2, space="PSUM")
            pqpool = qkps_ctx.__enter__()
            bps_ctx = tc.tile_pool(name="bps", bufs=2, space="PSUM")
            ps2pool = bps_ctx.__enter__()
            bpo_ctx = tc.tile_pool(name="bpo", bufs=1, space="PSUM")
            popool = bpo_ctx.__enter__()
            bpl_ctx = tc.tile_pool(name="bpl", bufs=1, space="PSUM")
            plpool = bpl_ctx.__enter__()
            st_ctx = tc.tile_pool(name="stw", bufs=4)
            stpool = st_ctx.__enter__()
            sw_ctx = tc.tile_pool(name="sww", bufs=4)
            swpool = sw_ctx.__enter__()

            def qkproj_chunks(h):
                """8 emission chunks: (k, nsl0), (q, nsl0), (k, nsl1), ...
                RoPE per [128, 512] chunk: cross-partition swaps + cos-mul
                on DVE, sin-mul + add on gpsimd (small ops so neither queue
                blocks the attention pipeline for long)."""
                chunks = []
                for nsl in range(4):
                    for wsb, bsb, dst in (
                        (wk_sb, bk_sb, kres),
                        (wq_sb, bq_sb, qres),
                    ):
                        def emit(nsl=nsl, wsb=wsb, bsb=bsb, dst=dst):
                            sl = slice(nsl * SC, (nsl + 1) * SC)
                            pq = pqpool.tile([128, SC], F32, tag="pq")
                            for k in range(KO):
                                nc.tensor.matmul(
                                    pq[:],
                                    wsb[:, k, h * DK : (h + 1) * DK],
                                    x_sb[:, nsl, k, :],
                                    start=(k == 0),
                                    stop=(k == KO - 1),
                                )
                            st = stpool.tile([128, SC], F16, tag="st")
                            nc.scalar.activation(
                                st[:], pq[:], AF.Identity,
                                bias=bsb[:, h : h + 1],
                            )
                            sw = swpool.tile([128, SC], F16, tag="sw")
                            nc.vector.tensor_copy(sw[0:64, :], st[64:128, :])
                            nc.vector.tensor_copy(sw[64:128, :], st[0:64, :])
                            nc.gpsimd.tensor_mul(sw[:], sw[:], sss_sb[:, sl])
                            nc.vector.tensor_mul(
                                dst[:, h, sl], st[:], cc2_sb[:, sl]
                            )
                            nc.gpsimd.tensor_add(
                                dst[:, h, sl], dst[:, h, sl], sw[:]
                            )
                        chunks.append(emit)
                return chunks

            tails = []  # deferred [ones-matmul, recip, normalize] per chunk

            def attn_chunks(h):
                """4 emission chunks, one per query chunk ic."""
                chunks = []
                for ic in range(NI):
                    def emit(ic=ic):
                        njc = 4 * ic + 4
                        i0 = ic * SC
                        po = popool.tile([128, SC], F32, tag="po")
                        acc = accpool.tile([128, SC], F16, tag="acc")
                        pend = []

                        def flush(wave, ws2):
                            p2 = p2pool.tile([128, 2, SC], F16, tag="p2")
                            if wave[-1][1] >= 0:
                                # diagonal band: exp only the valid columns
                                for j, (jc, t, cs) in enumerate(wave):
                                    nc.scalar.activation(
                                        p2[:, j, cs:], ws2[:, j, cs:],
                                        AF.Exp, scale=float(ISQRT_DK),
                                    )
                            else:
                                nc.scalar.activation(
                                    p2[:, 0:2, :], ws2[:, 0:2, :], AF.Exp,
                                    scale=float(ISQRT_DK),
                                )
                            for j, (jc, t, cs) in enumerate(wave):
                                if t >= 0:
                                    # in-tile causal triangle
                                    nc.vector.tensor_mul(
                                        p2[:, j, cs : cs + 128],
                                        p2[:, j, cs : cs + 128],
                                        tri_sb[:],
                                    )
                                if jc == 0:
                                    nc.scalar.copy(acc[:], p2[:, j, :])
                                else:
                                    nc.vector.tensor_add(
                                        acc[:, cs:], acc[:, cs:],
                                        p2[:, j, cs:],
                                    )
                            pend.append((p2, wave))

                        def drain_pv():
                            p2, wave = pend.pop(0)
                            for j, (jc, t, cs) in enumerate(wave):
                                nc.tensor.matmul(
                                    po[:, cs:],
                                    vt[:, jc, h * DK : (h + 1) * DK],
                                    p2[:, j, cs:],
                                    start=(jc == 0),
                                    stop=(jc == njc - 1),
                                )

                        wave, ws2 = [], None
                        first = True
                        for jc in range(njc):
                            t = jc - 4 * ic
                            cs = 128 * t if t >= 0 else 0
                            if not wave:
                                ws2 = ps2pool.tile([128, 2, SC], F32, tag="ps2")
                            nc.tensor.matmul(
                                ws2[:, len(wave), cs:],
                                kres[:, h, jc * 128 : (jc + 1) * 128],
                                qres[:, h, i0 + cs : i0 + SC],
                                start=True,
                                stop=True,
                            )
                            wave.append((jc, t, cs))
                            if len(wave) == 2:
                                flush(wave, ws2)
                                wave, ws2 = [], None
                                if first:
                                    # previous query chunk's softmax tail
                                    # lands here so its stragglers never
                                    # block this chunk's pipeline
                                    first = False
                                    if tails:
                                        tails.pop(0)()
                                if len(pend) > 1:
                                    drain_pv()
                        while pend:
                            drain_pv()

                        def tail(po=po, acc=acc, i0=i0):
                            pl = plpool.tile([128, SC], F32, tag="pl")
                            nc.tensor.matmul(
                                pl[:], ones_sb[:], acc[:],
                                start=True, stop=True,
                            )
                            li = lipool.tile([128, SC], F32, tag="li")
                            nc.vector.reciprocal_approx_fast(li[:], pl[:])
                            nc.vector.tensor_mul(
                                aores[:, h, i0 : i0 + SC], po[:], li[:]
                            )
                        tails.append(tail)
                    chunks.append(emit)
                return chunks

            # schedule: qkproj(0), then per head: attention(h) with
            # qkproj(h+1) chunks slotted after each query chunk
            for ch in qkproj_chunks(0):
                ch()
            # slot more projection chunks after the small early query chunks
            # and none after ic3: the rope DVE work then anti-correlates
            # with the growing softmax-accumulation burst
            slot = [(0, 3), (3, 6), (6, 8), (8, 8)]
            for h in range(HPC - 1):
                nxt = qkproj_chunks(h + 1)
                at = attn_chunks(h)
                for ic in range(NI):
                    at[ic]()
                    for ch in nxt[slot[ic][0] : slot[ic][1]]:
                        ch()

            # projection scratch done; swap for output weights, prefetched
            # during the last head's attention
            sw_ctx.__exit__(None, None, None)
            st_ctx.__exit__(None, None, None)
            wo_ctx = tc.tile_pool(name="wo", bufs=1)
            wopool = wo_ctx.__enter__()
            wo_sb = wopool.tile([128, HPC, D], F16, name="wo_sb")
            for g in range(4):
                (nc.sync if g % 2 == 0 else nc.gpsimd).dma_start(
                    wo_sb[:, g, :], woT[:, g, :]
                )
            for ch in attn_chunks(HPC - 1):
                ch()
            while tails:
                tails.pop(0)()

            bpl_ctx.__exit__(None, None, None)
            bpo_ctx.__exit__(None, None, None)
            bps_ctx.__exit__(None, None, None)
            qkps_ctx.__exit__(None, None, None)

            # ---------- Phase C: output projection (partial sums) ----------
            with (
                tc.tile_pool(name="cob", bufs=2) as obpool,
                tc.tile_pool(name="cps", bufs=2, space="PSUM") as cpspool,
            ):
                for ii in range(S // 128):
                    isl = slice(ii * 128, (ii + 1) * 128)
                    ob = obpool.tile([128, D], F16, tag="ob")
                    for half in range(2):
                        pc = cpspool.tile([128, 2, 512], F32, tag="pc")
                        for ec in range(HPC):
                            for f2 in range(2):
                                fc = half * 2 + f2
                                nc.tensor.matmul(
                                    pc[:, f2, :],
                                    aores[:, ec, isl],
                                    wo_sb[:, ec, fc * 512 : (fc + 1) * 512],
                                    start=(ec == 0),
                                    stop=(ec == HPC - 1),
                                )
                        for f2 in range(2):
                            fc = half * 2 + f2
                            osl = slice(fc * 512, (fc + 1) * 512)
                            if f2 == 0:
                                nc.vector.tensor_copy(ob[:, osl], pc[:, f2, :])
                            else:
                                nc.scalar.activation(
                                    ob[:, osl], pc[:, f2, :], AF.Copy
                                )
                    (nc.sync if ii % 2 == 0 else nc.scalar).dma_start(
                        out[isl, :], ob[:]
                    )
            wo_ctx.__exit__(None, None, None)

    nc.compile()
    return nc


def _rope_tables():
    inv_freq = 1.0 / (10000.0 ** (np.arange(0, DK, 2, dtype=np.float64) / DK))
    pos = np.arange(S, dtype=np.float64)
    freqs = pos[:, None] * inv_freq[None, :]  # [S, DK/2]
    cos_t = np.cos(freqs).T.astype(np.float16)  # [64, S]
    sin_t = np.sin(freqs).T.astype(np.float16)
    cc2 = np.ascontiguousarray(np.concatenate([cos_t, cos_t], axis=0))
    sss = np.ascontiguousarray(np.concatenate([-sin_t, sin_t], axis=0))
    return cc2, sss


def _pack_pke(w16):
    """[D, E] -> [128, KO, E] with partition dim first, contiguous."""
    return np.ascontiguousarray(
        w16.reshape(KO, 128, E).transpose(1, 0, 2)
    )


def kernel(
    x, wq_w, wq_b, wk_w, wk_b, wv_w, wv_b, wo_w, wo_b
) -> np.ndarray:
    global last_exec_time_ns, last_results
    from concourse.bass_utils import run_bass_kernel_spmd

    if "nc" not in _CACHE:
        _CACHE["nc"] = _build_program()
    nc = _CACHE["nc"]

    x = np.asarray(x, dtype=np.float32)
    wq_w = np.asarray(wq_w, dtype=np.float32)
    wk_w = np.asarray(wk_w, dtype=np.float32)
    wv_w = np.asarray(wv_w, dtype=np.float32)
    wo_w = np.asarray(wo_w, dtype=np.float32)
    wq_b = np.asarray(wq_b, dtype=np.float32)
    wk_b = np.asarray(wk_b, dtype=np.float32)
    wv_b = np.asarray(wv_b, dtype=np.float32)
    wo_b = np.asarray(wo_b, dtype=np.float32)

    cc2, sss = _rope_tables()
    r_idx = np.arange(128)[:, None]
    c_idx = np.arange(128)[None, :]
    tri = np.ascontiguousarray((r_idx <= c_idx).astype(np.float16))
    ones = np.ones((128, 128), dtype=np.float16)
    # within each head, pack d-rows as [even dims; odd dims]
    perm = np.concatenate([np.arange(0, DK, 2), np.arange(1, DK, 2)])

    # x: [S, D] -> xT [D, S] -> [128, NI(si), KO(k), SC] contiguous
    xT_b = [
        np.ascontiguousarray(
            x[b].T.astype(np.float16)
            .reshape(KO, 128, NI, SC)
            .transpose(1, 2, 0, 3)
        )
        for b in range(B)
    ]

    in_maps = []
    for c in range(N_CORES):
        b = c // (N_CORES // B)
        g = c % (N_CORES // B)
        es = g * E

        def pack_qk(w):
            rows = w[es : es + E]  # [E, D]
            blocks = [
                rows[h0 * DK : (h0 + 1) * DK][perm] for h0 in range(HPC)
            ]
            return _pack_pke(
                np.concatenate(blocks, axis=0).T.astype(np.float16)
            )

        def pack_bias(bvec):
            sl = bvec[es : es + E].reshape(HPC, DK)
            return np.ascontiguousarray(sl[:, perm].T)  # [DK, HPC]

        in_maps.append(
            {
                "xT": xT_b[b],
                "wqT": pack_qk(wq_w),
                "wkT": pack_qk(wk_w),
                "wvT": _pack_pke(wv_w[es : es + E].T.astype(np.float16)),
                "woT": np.ascontiguousarray(
                    wo_w[:, es : es + E].T.astype(np.float16)
                    .reshape(HPC, 128, D)
                    .transpose(1, 0, 2)
                ),
                "bq": pack_bias(wq_b),
                "bk": pack_bias(wk_b),
                "cc2": cc2,
                "sss": sss,
                "tri": tri,
                "ones": ones,
            }
        )

    trace = bool(os.environ.get("MHA_TRACE"))
    res = run_bass_kernel_spmd(
        nc, in_maps, list(range(N_CORES)), trace=trace
    )
    last_exec_time_ns = res.exec_time_ns
    last_results = res

    # host-side gather: sum partials per batch, add biases that commute
    # with attention (softmax rows sum to 1, so wv_b passes straight
    # through to the output projection)
    const_bias = wo_b + wo_w @ wv_b  # [D]
    out = np.empty((B, S, D), dtype=np.float32)
    gpb = N_CORES // B
    for b in range(B):
        acc = res.results[b * gpb]["out"].astype(np.float32)
        for c in range(b * gpb + 1, (b + 1) * gpb):
            acc += res.results[c]["out"].astype(np.float32)
        out[b] = acc + const_bias[None, :]
    return out
